# revision 33
# baseline (speedup 1.0000x reference)
"""Trainium2 Bass kernel for causal self-attention with cumulative-phase rotary
embedding (nn_CausalSelfAttention_64338610094602).

Sharding: 8 cores = 4 batches x 2 head-groups (tensor-parallel over heads).
Each core computes, for its (batch, 8-head group):
  omega/phi (replicated per batch), QKV projections, rotation + RMSNorm,
  causal attention (transposed-scores layout, max-free softmax), and a
  partial output projection. Host sums the two head-group partials per batch.

v5 design notes (vs v4's per-pair phases):
  - All projections first (P1 omega/trig, P2 all 4 pairs' q/k/v), then one
    flat attention pipeline over all 32 (head, J) block-rows, then P4.
    The PE instruction stream never alternates sections, which avoids both
    the per-row ACT-latency bubbles and the p-state ramp (PE runs at 1.2GHz
    for 3us after any idle gap, 2.4GHz only when continuously busy).
  - q/k (all 8 heads, post-norm, bf16) spill to DRAM during P2 and stream
    back per-head in P3 (SBUF cannot hold 8 heads of q+k next to xts);
    v and y stay SBUF-resident for all heads (no y round-trip).
  - Softmax denominator: each ex2 [128,1024] tile is folded to [128,512]
    on DVE (bf16 add of the two key-tile halves) and the PE ones-matmul
    runs on the folded tile -- half the PE columns of v4's dps.
  - Rotation sign baked into the frequency vector (rows 64:128 negative)
    so trig tiles are written straight out of ACT Sin; gamma applied in the
    RMSNorm multiply (scalar_tensor_tensor) instead of folded into trig.
  - Causal mask folded into the PE score accumulation (trilA x maskB adds
    -1e9*count on diagonal tiles) as in v4.
  - All 4 Wo column blocks prefetched into SBUF during P3; P4 reads y_sb
    directly, so the P3->P4 transition has no DMA wait.
"""
import math

import numpy as np
import ml_dtypes

import concourse.mybir as mybir
import concourse.tile as tile
from concourse import bass_isa
from concourse import bacc
from concourse.bass_utils import run_bass_kernel_spmd

B, T, C = 4, 2048, 2048
H, D, DH = 16, 128, 64
HG = 8          # heads per core (head-group)
GD = HG * D     # group output dims = 1024
NT = T // 512   # 4 query blocks of 512
NCT = C // 128  # 16 contraction tiles
EPS = 1e-5
SCL = 1.0 / math.sqrt(D)
NEG = -1.0e9

dt = mybir.dt
AF = mybir.ActivationFunctionType
ALU = mybir.AluOpType

TWO_PI = 6.283185307179586
INV_2PI = 1.0 / TWO_PI
CW1 = float(np.float32(6.28125))
CW2 = float(np.float32(TWO_PI - 6.28125))
CW3 = float(TWO_PI - CW1 - float(np.float32(TWO_PI - 6.28125)))
MAGIC = 12582912.0  # 1.5 * 2^23: fp32 add/sub rounds to nearest int
HALF_PI = 1.5707963267948966
PI = 3.141592653589793

_CACHE = {}


def _build():
    f32, bf16 = dt.float32, dt.bfloat16
    nc = bacc.Bacc(None, target_bir_lowering=False)
    with tile.TileContext(nc) as tc:
        # weight inputs are host-prearranged to the exact SBUF layouts so
        # every DMA moves 4KB-contiguous per-partition runs
        xt_d = nc.dram_tensor("xt", (C, T), bf16, kind="ExternalInput")
        wq_d = nc.dram_tensor("wq", (HG, 128, NCT * 128), bf16,
                              kind="ExternalInput")
        wk_d = nc.dram_tensor("wk", (HG, 128, NCT * 128), bf16,
                              kind="ExternalInput")
        wv_d = nc.dram_tensor("wv", (4, 128, NCT * 256), bf16,
                              kind="ExternalInput")
        wo_d = nc.dram_tensor("wo", (128, 4 * HG * 512), bf16,
                              kind="ExternalInput")
        womg2_d = nc.dram_tensor("womg2", (128, NCT * 128), bf16,
                                 kind="ExternalInput")
        b16_d = nc.dram_tensor("b16", (1, 1), f32, kind="ExternalInput")
        freqs_d = nc.dram_tensor("freqs", (128, 1), f32, kind="ExternalInput")
        gq_d = nc.dram_tensor("gq", (128, 1), f32, kind="ExternalInput")
        gk_d = nc.dram_tensor("gk", (128, 1), f32, kind="ExternalInput")
        maskB_d = nc.dram_tensor("maskB", (128, 4 * 512), bf16, kind="ExternalInput")
        ones128_d = nc.dram_tensor("ones128", (128, 128), bf16,
                                   kind="ExternalInput")
        out_d = nc.dram_tensor("out", (T, C), f32, kind="ExternalOutput")

        with tc.tile_pool(name="const", bufs=1) as constp, \
             tc.tile_pool(name="dram", bufs=1, space="DRAM") as dramp, \
             tc.tile_pool(name="core", bufs=1) as corep, \
             tc.tile_pool(name="qkp", bufs=1) as qkp, \
             tc.tile_pool(name="psp", bufs=1, space="PSUM") as psp:

            # ---- constants ----
            b16t = constp.tile([1, 1], f32)
            nc.sync.dma_start(b16t[:], b16_d[:])
            freqs = constp.tile([128, 1], f32)
            nc.sync.dma_start(freqs[:], freqs_d[:])
            gq = constp.tile([128, 1], f32)
            nc.sync.dma_start(gq[:], gq_d[:])
            gk = constp.tile([128, 1], f32)
            nc.sync.dma_start(gk[:], gk_d[:])
            maskB = constp.tile([128, 4 * 512], bf16)
            ones128 = constp.tile([128, 128], bf16)
            nc.sync.dma_start(ones128[:], ones128_d[:])
            eps128 = constp.tile([128, 1], f32)
            nc.vector.memset(eps128[:], EPS)

            # all-heads v and y stay resident; q/k spill to DRAM (separate
            # tiles so a head's readback only waits on its own spill)
            v_sb = corep.tile([128, 4 * 16 * 256], bf16)  # (pair*16+tt)*256
            y_sb = corep.tile([128, HG * T], bf16)        # yT per head at h*T
            qk_d = {(wi, h): dramp.tile([128, T], bf16, name=f"qkd_{wi}_{h}")
                    for wi in range(2) for h in range(HG)}

            # stream q/k per head (ring 2); heads 0/1 are fetched from
            # inside P2 as soon as their spills are issued
            qh_slots = [None, None]

            def fetch_head(h):
                qh = qkp.tile([128, T], bf16, tag="qh", bufs=2,
                              name=f"qh_{h}")
                kh = qkp.tile([128, T], bf16, tag="kh", bufs=2,
                              name=f"kh_{h}")
                for c in range(2):
                    sl = slice(c * 1024, (c + 1) * 1024)
                    nc.sync.dma_start(qh[:, sl], qk_d[(0, h)][:, sl])
                    nc.sync.dma_start(kh[:, sl], qk_d[(1, h)][:, sl])
                qh_slots[h % 2] = (qh, kh)

            with tc.tile_pool(name="xtp", bufs=1) as xtp, \
                 tc.tile_pool(name="wstp", bufs=1) as wstp, \
                 tc.tile_pool(name="trigp", bufs=1) as trigp:
                trigA = trigp.tile([128, T], bf16)
                trigB = trigp.tile([128, T], bf16)
                _proj(nc, tc, xt_d, wq_d, wk_d, wv_d, womg2_d,
                      xtp, wstp, psp,
                      b16t, freqs, gq, gk, ones128, eps128,
                      trigA, trigB, v_sb, qk_d, fetch_head)

            with tc.tile_pool(name="attp", bufs=1) as attp, \
                 tc.tile_pool(name="p4w", bufs=1) as p4w, \
                 tc.tile_pool(name="p4o", bufs=1) as p4o:
                for c in range(2):
                    nc.sync.dma_start(maskB[:, c * 1024:(c + 1) * 1024],
                                      maskB_d[:, c * 1024:(c + 1) * 1024])
                wo_all = p4w.tile([128, 4 * HG * 512], bf16)  # (cb*8+hh)*512
                for cb in range(4):
                    for c in range(2):
                        sl = slice(cb * 4096 + c * 2048,
                                   cb * 4096 + (c + 1) * 2048)
                        nc.sync.dma_start(wo_all[:, sl], wo_d[:, sl])

                _attention(nc, tc, attp, psp, qh_slots, fetch_head,
                           maskB, ones128, v_sb, y_sb)

                # ---- P4: out = y^T W_o (partial over heads) ----
                for ti in range(T // 128):
                    for cb in range(4):
                        ops = psp.tile([128, 512], f32, tag="y", bufs=4,
                                       name=f"ops_{ti}_{cb}")
                        for hh in range(HG):
                            nc.tensor.matmul(
                                ops[:],
                                y_sb[:, hh * T + ti * 128:hh * T + (ti + 1) * 128],
                                wo_all[:, (cb * 8 + hh) * 512:(cb * 8 + hh + 1) * 512],
                                start=(hh == 0), stop=(hh == HG - 1))
                        osb = p4o.tile([128, 512], f32, tag="osb", bufs=4)
                        if cb % 2 == 0:
                            nc.scalar.copy(osb[:], ops[:])
                        else:
                            nc.vector.tensor_copy(osb[:], ops[:])
                        nc.sync.dma_start(
                            out_d[ti * 128:(ti + 1) * 128,
                                  cb * 512:(cb + 1) * 512],
                            osb[:])
    nc.compile()
    return nc


def _proj(nc, tc, xt_d, wq_d, wk_d, wv_d, womg2_d,
          xtp, wstp, psp,
          b16t, freqs, gq, gk, ones128, eps128,
          trigA, trigB, v_sb, qk_d, fetch_head):
    f32, bf16 = dt.float32, dt.bfloat16

    sites = [(pair, wi, hl) for pair in range(4) for wi in range(2)
             for hl in range(2)]
    wp_slots = [None, None]
    wvp_slots = [None]

    # each dma_start lands on one ~22GB/s queue: split panel transfers into
    # chunks so they spread across queues (runs stay 4KB-contiguous)
    def issue_panel(si):
        pair, wi, hl = sites[si]
        h = pair * 2 + hl
        w_d = (wq_d, wk_d)[wi]
        wp = wstp.tile([128, NCT * 128], bf16, tag="wp", bufs=2,
                       name=f"wp_{si}")
        for c in range(2):
            nc.sync.dma_start(wp[:, c * 1024:(c + 1) * 1024],
                              w_d[h, :, c * 1024:(c + 1) * 1024])
        wp_slots[si % 2] = wp

    def issue_wvp(pair):
        wvp = wstp.tile([128, NCT * 256], bf16, tag="wvp", bufs=1,
                        name=f"wvp_{pair}")
        for c in range(4):
            nc.sync.dma_start(wvp[:, c * 1024:(c + 1) * 1024],
                              wv_d[pair, :, c * 1024:(c + 1) * 1024])
        wvp_slots[0] = wvp

    # ---- P1: omega -> phi -> trig (pools closed before P2's scratch) ----
    # split by T-halves so trig for J0/J1 is ready as soon as the first
    # half of x lands; the x DMA is half-major for the same reason
    with tc.tile_pool(name="p1p", bufs=1) as p1p, \
         tc.tile_pool(name="rowp", bufs=1) as rowp:
        womg2 = p1p.tile([128, NCT * 128], bf16, name="womg2")
        for c in range(8):
            nc.sync.dma_start(womg2[:, c * 256:(c + 1) * 256],
                              womg2_d[:, c * 256:(c + 1) * 256])
        xts = xtp.tile([128, NCT * T], bf16)  # c-tile i at [i*T,(i+1)*T)
        for half in range(2):
            for i in range(NCT):
                for c2 in range(2):
                    cs = half * 1024 + c2 * 512
                    nc.sync.dma_start(
                        xts[:, i * T + cs:i * T + cs + 512],
                        xt_d[i * 128:(i + 1) * 128, cs:cs + 512])
            if half == 0:
                issue_panel(0)
        issue_wvp(0)

        HT = T // 2
        omega = rowp.tile([1, T], f32, tag="om")
        incl = rowp.tile([1, T], f32, tag="incl")
        off = rowp.tile([1, 1], f32, tag="off")

        def trig_J(J):
            sl = slice(J * 512, (J + 1) * 512)
            phi2 = p1p.tile([128, 512], f32, tag="p1", bufs=3,
                            name=f"phi2_{J}")
            nc.gpsimd.partition_broadcast(phi2[:], incl[:, sl])
            ang = p1p.tile([128, 512], f32, tag="p1", bufs=3, name=f"ang_{J}")
            # rows 64:128 of freqs are negated: sin rows come out negated,
            # cos rows unchanged (even), which is the rotation's sign layout
            nc.vector.tensor_scalar(ang[:], phi2[:], freqs[:], None,
                                    op0=ALU.mult)
            mm = p1p.tile([128, 512], f32, tag="p1", bufs=3, name=f"mm_{J}")
            nc.vector.tensor_scalar(mm[:], ang[:], INV_2PI, MAGIC,
                                    op0=ALU.mult, op1=ALU.add)
            kk = p1p.tile([128, 512], f32, tag="p1", bufs=3, name=f"kk_{J}")
            nc.vector.tensor_scalar_add(kk[:], mm[:], -MAGIC)
            red = p1p.tile([128, 512], f32, tag="p1", bufs=3, name=f"red_{J}")
            nc.vector.cody_waite_cascade(red[:], ang[:], kk[:], CW1, CW2, CW3)
            red2 = p1p.tile([128, 512], f32, tag="p1", bufs=3,
                            name=f"red2_{J}")
            nc.vector.add_range_wrap(red2[:], red[:], HALF_PI, PI, TWO_PI)
            nc.scalar.activation(trigB[:, sl], red[:], AF.Sin)
            nc.scalar.activation(trigA[:, sl], red2[:], AF.Sin)

        for half in range(2):
            hsl = slice(half * HT, (half + 1) * HT)
            for Jh in range(2):
                J = half * 2 + Jh
                omps = psp.tile([128, 512], f32, tag="y", bufs=4,
                                name=f"omps_{J}")
                for i in range(NCT):
                    nc.tensor.matmul(
                        omps[:], womg2[:, i * 128:(i + 1) * 128],
                        xts[:, i * T + J * 512:i * T + J * 512 + 512],
                        start=(i == 0), stop=(i == NCT - 1))
                nc.scalar.activation(omega[:, J * 512:(J + 1) * 512],
                                     omps[0:1, :],
                                     AF.Sigmoid, scale=1.0 / 16.0,
                                     bias=b16t[:])
            # inclusive scan of this half, then phi (in-place) = incl - omega
            nc.vector.tensor_tensor_scan(incl[:, hsl], omega[:, hsl],
                                         omega[:, hsl], 0.0,
                                         ALU.add, ALU.bypass)
            if half == 0:
                nc.vector.tensor_copy(off[:], incl[:, HT - 1:HT])
            else:
                nc.vector.tensor_scalar(incl[:, hsl], incl[:, hsl],
                                        off[:], None, op0=ALU.add)
            nc.vector.tensor_sub(incl[:, hsl], incl[:, hsl], omega[:, hsl])
            trig_J(half * 2)
            trig_J(half * 2 + 1)

    # ---- P2: q/k/v for all pairs; q/k rotated+normed then spilled ----
    pend_norm = [None]
    pend_tail = [None]

    def flush(pend):
        if pend[0] is not None:
            pend[0]()
            pend[0] = None

    with tc.tile_pool(name="scp", bufs=1) as scp:
        for pair in range(4):
            wvp = wvp_slots[0]

            # --- v first: needs no trig, so the P1 sigmoid->scan->trig
            # chain has cover before the first rotation consumer ---
            vbase = pair * 16 * 256
            for tq in range(4):
                vps = []
                for q4 in range(2):
                    vps.append(psp.tile([128, 1024], f32, tag="s", bufs=2,
                                        name=f"vps_{pair}_{tq}_{q4}"))
                for q4 in range(2):
                    for i in range(NCT):
                        for t2 in range(2):
                            t = q4 * 2 + t2
                            tt = tq * 4 + t
                            nc.tensor.matmul(
                                vps[q4][:, t2 * 512:t2 * 512 + 256],
                                xts[:, i * T + tt * 128:i * T + (tt + 1) * 128],
                                wvp[:, i * 256:(i + 1) * 256],
                                start=(i == 0), stop=(i == NCT - 1))
                for t in range(4):
                    tt = tq * 4 + t
                    # split copies ACT/DVE so neither engine's backlog
                    # stalls vps PSUM-bank reuse
                    dst = v_sb[:, vbase + tt * 256:vbase + (tt + 1) * 256]
                    src = vps[t // 2][:, (t % 2) * 512:(t % 2) * 512 + 256]
                    if t % 2 == 0 and pair < 3:
                        nc.scalar.copy(dst, src)
                    else:
                        nc.vector.tensor_copy(dst, src)
                if tq == 0:
                    flush(pend_tail)
                    flush(pend_norm)
                    if pair == 1:
                        # pair-0 spills (heads 0/1) are all issued now
                        fetch_head(0)
                        fetch_head(1)
            if pair + 1 < 4:
                issue_wvp(pair + 1)

            for wi in range(2):
                for hl in range(2):
                    si = pair * 4 + wi * 2 + hl
                    if si + 1 < len(sites):
                        issue_panel(si + 1)
                    wp = wp_slots[si % 2]
                    h = pair * 2 + hl
                    spill_d = qk_d[(wi, h)]
                    g = (gq, gk)[wi]
                    qsite = scp.tile([128, T], bf16, tag="qk", bufs=2,
                                     name=f"qsite_{si}")
                    sqs = []
                    for Jp in range(2):
                        qps2 = psp.tile([128, 1024], f32, tag="s", bufs=2,
                                        name=f"qps2_{si}_{Jp}")
                        for i in range(NCT):
                            for Jh in range(2):
                                J = 2 * Jp + Jh
                                nc.tensor.matmul(
                                    qps2[:, Jh * 512:(Jh + 1) * 512],
                                    wp[:, i * 128:(i + 1) * 128],
                                    xts[:, i * T + J * 512:i * T + J * 512 + 512],
                                    start=(i == 0), stop=(i == NCT - 1))
                        # flush prev site's ssq tail mid-stream so its rnb
                        # is ready before this site's norm
                        if Jp == 1:
                            flush(pend_tail)
                        for Jh in range(2):
                            J = 2 * Jp + Jh
                            qps = qps2[:, Jh * 512:(Jh + 1) * 512]
                            sl = slice(J * 512, (J + 1) * 512)
                            # rotation: cos part straight into qsite, then
                            # += swapped-half sin part (sign baked in trigB)
                            nc.vector.tensor_tensor(qsite[:, sl], qps,
                                                    trigA[:, sl], op=ALU.mult)
                            Bt = scp.tile([128, 512], f32, tag="rb", bufs=2,
                                          name=f"Bt_{si}_{J}")
                            nc.vector.tensor_tensor(
                                Bt[0:DH, :],
                                qps2[DH:128, Jh * 512:(Jh + 1) * 512],
                                trigB[0:DH, sl], op=ALU.mult)
                            nc.vector.tensor_tensor(
                                Bt[DH:128, :],
                                qps2[0:DH, Jh * 512:(Jh + 1) * 512],
                                trigB[DH:128, sl], op=ALU.mult)
                            nc.vector.tensor_add(
                                qsite[:, sl], qsite[:, sl], Bt[:])
                            # sum-of-squares (rotation preserves norms)
                            sq = scp.tile([128, 512], bf16, tag="sq", bufs=6,
                                          name=f"sq_{si}_{J}")
                            nc.scalar.activation(sq[:], qps, AF.Square)
                            sqs.append((J, sq))
                    flush(pend_norm)

                    def tail(sqs=tuple(sqs), si=si, qsite=qsite, g=g,
                             spill_d=spill_d, pend_norm=pend_norm):
                        rnbs = []
                        for J, sq in sqs:
                            ssqps = psp.tile([128, 512], f32, tag="y", bufs=4,
                                             name=f"ssq_{si}_{J}")
                            nc.tensor.matmul(ssqps[:], ones128[:], sq[:],
                                             start=True, stop=True)
                            rnb = scp.tile([128, 512], bf16, tag="rnb",
                                           bufs=4, name=f"rnb_{si}_{J}")
                            nc.scalar.activation(rnb[:], ssqps[:],
                                                 AF.Abs_reciprocal_sqrt,
                                                 scale=1.0 / 128.0,
                                                 bias=eps128[:])
                            rnbs.append((J, rnb))

                        def norm():
                            for J, rnb in rnbs:
                                sl = slice(J * 512, (J + 1) * 512)
                                nc.vector.scalar_tensor_tensor(
                                    qsite[:, sl], qsite[:, sl], g[:], rnb[:],
                                    op0=ALU.mult, op1=ALU.mult)
                            nc.sync.dma_start(spill_d[:], qsite[:])
                        pend_norm[0] = norm
                    pend_tail[0] = tail

        flush(pend_tail)
        flush(pend_norm)


def _attention(nc, tc, attp, psp, qh_slots, fetch_head,
               maskB, ones128, v_sb, y_sb):
    """Flat software pipeline over all (h, J) block-rows at Ip granularity.

    Per task (h, J, Ip): scores for key-tile pair Ip into a [128,1024] PSUM
    tile, ACT Exp -> ex2 bf16, 0/1 mask multiply on diagonal tiles (DVE),
    and a two-level DVE fold tree feeding a GpSimd partition_all_reduce +
    accumulate for the softmax denominator (no PE involvement).  Consumption
    lags 2 tasks: yps matmuls per ex2 half.  Row epilogue (reciprocal of the
    GpSimd-reduced denominator + y write) runs on DVE.
    """
    f32, bf16 = dt.float32, dt.bfloat16
    tasks = []
    for h in range(HG):
        # J descending: the first tasks of each head are non-diagonal, so
        # the pipeline fill never waits on the DVE mask path
        for J in reversed(range(NT)):
            for Ip in range(2 * J + 2):
                tasks.append((h, J, Ip))

    state = {}  # (h, J) -> (yps, dps)
    pend_fold = [None]
    inflight = []

    def issue(ti_t):
        ti, t = ti_t
        h, J, Ip = t
        if J == NT - 1 and Ip == 0 and 1 <= h < HG - 1:
            # heads 0/1 are prefetched from P2; ring slot h-1 frees once
            # all of head h-1's scores have issued
            fetch_head(h + 1)
        qh, kh = qh_slots[h % 2]
        sps2 = psp.tile([128, 1024], f32, tag="s", bufs=2,
                        name=f"sps_{h}_{J}_{Ip}")
        for half in range(2):
            I = 2 * Ip + half
            osl = sps2[:, half * 512:(half + 1) * 512]
            nc.tensor.matmul(
                osl,
                kh[:, I * 128:(I + 1) * 128],
                qh[:, J * 512:(J + 1) * 512],
                start=True, stop=True)
        ex2 = attp.tile([128, 1024], bf16, tag="ex", bufs=4,
                        name=f"ex_{h}_{J}_{Ip}")
        diag_r = 2 * Ip - 4 * J
        if diag_r == 2 and ti >= 4:
            # second diagonal tile: columns [0,256) are fully masked; skip
            # their exp.  The stale ring-slot contents there are old finite
            # exp values (ti>=4 skips first use), zeroed by the mask below.
            nc.scalar.activation(ex2[:, 256:1024], sps2[:, 256:1024],
                                 AF.Exp, scale=SCL)
        else:
            nc.scalar.activation(ex2[:], sps2[:], AF.Exp, scale=SCL)
        if diag_r >= 0:
            # causal mask: zero the upper-triangular part of the two
            # diagonal key tiles with one in-place 0/1 multiply (DVE)
            nc.vector.tensor_tensor(ex2[:], ex2[:],
                                    maskB[:, diag_r * 512:diag_r * 512 + 1024],
                                    op=ALU.mult)
        fold = attp.tile([128, 512], bf16, tag="fold", bufs=4,
                         name=f"fold_{h}_{J}_{Ip}")
        nc.vector.tensor_add(fold[:], ex2[:, 0:512], ex2[:, 512:1024])
        if Ip % 2 == 0:
            pend_fold[0] = fold
            dps_op = None
        else:
            # second fold level: one dps matmul per 4 key tiles
            dps_op = attp.tile([128, 512], bf16, tag="fold2", bufs=3,
                               name=f"fold2_{h}_{J}_{Ip}")
            nc.vector.tensor_add(dps_op[:], pend_fold[0][:], fold[:])
        return (t, ex2, dps_op)

    def consume(item):
        t, ex2, dps_op = item
        h, J, Ip = t
        nI = 4 * J + 4
        nIp = 2 * J + 2
        if Ip == 0:
            yps = psp.tile([128, 512], f32, tag="y", bufs=4,
                           name=f"yps_{h}_{J}")
            dps = psp.tile([128, 512], f32, tag="y", bufs=4,
                           name=f"dps_{h}_{J}")
            state[(h, J)] = (yps, dps)
        yps, dps = state[(h, J)]
        vbase = (h // 2) * 16 * 256
        hoff = (h % 2) * 128
        for half in range(2):
            I = 2 * Ip + half
            nc.tensor.matmul(
                yps[:],
                v_sb[:, vbase + I * 256 + hoff:vbase + I * 256 + hoff + 128],
                ex2[:, half * 512:(half + 1) * 512],
                start=(I == 0), stop=(I == nI - 1))
        if dps_op is not None:
            nc.tensor.matmul(dps[:], ones128[:], dps_op[:],
                             start=(Ip == 1), stop=(Ip == nIp - 1))
        if Ip == nIp - 1:
            rb = attp.tile([128, 512], f32, tag="rbc", bufs=2,
                           name=f"rb_{h}_{J}")
            nc.vector.reciprocal_approx_fast(out=rb[:], in_=dps[:])
            nc.vector.tensor_tensor(
                y_sb[:, h * T + J * 512:h * T + (J + 1) * 512],
                yps[:], rb[:], op=ALU.mult)
            del state[(h, J)]

    LAG = 2
    for ti, t in enumerate(tasks):
        if len(inflight) >= LAG:
            consume(inflight.pop(0))
        inflight.append(issue((ti, t)))
    while inflight:
        consume(inflight.pop(0))


def _host_prep(inputs):
    bf = ml_dtypes.bfloat16
    x = np.asarray(inputs["x"], dtype=np.float32)
    Wq = np.asarray(inputs["Wq"], dtype=np.float32)
    Wk = np.asarray(inputs["Wk"], dtype=np.float32)
    Wv = np.asarray(inputs["Wv"], dtype=np.float32)
    Wo = np.asarray(inputs["Wo"], dtype=np.float32)
    w_omega = np.asarray(inputs["w_omega"], dtype=np.float32)
    b_omega = np.asarray(inputs["b_omega"], dtype=np.float32)
    log_freq = np.asarray(inputs["log_freq"], dtype=np.float32)
    q_gamma = np.asarray(inputs["q_gamma"], dtype=np.float32)
    k_gamma = np.asarray(inputs["k_gamma"], dtype=np.float32)

    womg = w_omega.reshape(NCT, 128).T.astype(np.float32)
    # replicated across output rows: womg2[:, i*128+c] = w_omega[i*128+:] col c
    womg2 = np.repeat(womg.T[:, :, None], 128, axis=2)  # [i, 128k, 128c]
    womg2 = womg2.transpose(1, 0, 2).reshape(128, NCT * 128).astype(bf)
    b16 = (b_omega / 16.0).reshape(1, 1).astype(np.float32)
    f = np.exp(log_freq)
    freqs = np.concatenate([f, -f]).reshape(128, 1).astype(np.float32)
    gqv = q_gamma.reshape(128, 1).astype(np.float32)
    gkv = k_gamma.reshape(128, 1).astype(np.float32)
    ones128 = np.ones((128, 128), dtype=bf)
    p = np.arange(128)[:, None]
    c = np.arange(512)[None, :]
    # 0/1 keep-mask for the diagonal key tiles: key p + r*128 <= query c
    maskB = np.concatenate(
        [((p + r * 128) <= c).astype(np.float32) for r in range(4)],
        axis=1).astype(bf)

    def panels_qk(W, g):
        # [h, p, i*128+m] = W_core_T[i*128+p, h*128+m]
        WT = W[g * GD:(g + 1) * GD, :].T  # [C, GD]
        A = WT.reshape(NCT, 128, HG, 128).transpose(2, 1, 0, 3)
        return np.ascontiguousarray(A.reshape(HG, 128, NCT * 128)).astype(bf)

    def panels_v(W, g):
        # [pair, p, i*256+n] = W_core_T[i*128+p, pair*256+n]
        WT = W[g * GD:(g + 1) * GD, :].T
        A = WT.reshape(NCT, 128, 4, 256).transpose(2, 1, 0, 3)
        return np.ascontiguousarray(A.reshape(4, 128, NCT * 256)).astype(bf)

    def panel_o(W, g):
        # [p, (cb*8+hh)*512+c] = W_core_T[hh*128+p, cb*512+c]
        WT = W[:, g * GD:(g + 1) * GD].T  # [GD, C]
        A = WT.reshape(HG, 128, 4, 512).transpose(1, 2, 0, 3)
        return np.ascontiguousarray(A.reshape(128, 4 * HG * 512)).astype(bf)

    in_maps = []
    for core in range(8):
        b, g = core // 2, core % 2
        in_maps.append({
            "xt": np.ascontiguousarray(x[b].T).astype(bf),
            "wq": panels_qk(Wq, g),
            "wk": panels_qk(Wk, g),
            "wv": panels_v(Wv, g),
            "wo": panel_o(Wo, g),
            "womg2": womg2, "b16": b16,
            "freqs": freqs,
            "gq": gqv, "gk": gkv,
            "maskB": maskB, "ones128": ones128,
        })
    return in_maps


def kernel(**inputs) -> np.ndarray:
    if "nc" not in _CACHE:
        _CACHE["nc"] = _build()
    nc = _CACHE["nc"]
    in_maps = _host_prep(inputs)
    res = run_bass_kernel_spmd(nc, in_maps, core_ids=list(range(8)))
    out = np.empty((B, T, C), dtype=np.float32)
    for b in range(B):
        out[b] = res.results[2 * b]["out"] + res.results[2 * b + 1]["out"]
    return out


# revision 34
# speedup vs baseline: 1.0196x; 1.0196x over previous
"""Trainium2 Bass kernel for causal self-attention with cumulative-phase rotary
embedding (nn_CausalSelfAttention_64338610094602).

Sharding: 8 cores = 4 batches x 2 head-groups (tensor-parallel over heads).
Each core computes, for its (batch, 8-head group):
  omega/phi (replicated per batch), QKV projections, rotation + RMSNorm,
  causal attention (transposed-scores layout, max-free softmax), and a
  partial output projection. Host sums the two head-group partials per batch.

v5 design notes (vs v4's per-pair phases):
  - All projections first (P1 omega/trig, P2 all 4 pairs' q/k/v), then one
    flat attention pipeline over all 32 (head, J) block-rows, then P4.
    The PE instruction stream never alternates sections, which avoids both
    the per-row ACT-latency bubbles and the p-state ramp (PE runs at 1.2GHz
    for 3us after any idle gap, 2.4GHz only when continuously busy).
  - q/k (all 8 heads, post-norm, bf16) spill to DRAM during P2 and stream
    back per-head in P3 (SBUF cannot hold 8 heads of q+k next to xts);
    v and y stay SBUF-resident for all heads (no y round-trip).
  - Softmax denominator: each ex2 [128,1024] tile is folded to [128,512]
    on DVE (bf16 add of the two key-tile halves) and the PE ones-matmul
    runs on the folded tile -- half the PE columns of v4's dps.
  - Rotation sign baked into the frequency vector (rows 64:128 negative)
    so trig tiles are written straight out of ACT Sin; gamma applied in the
    RMSNorm multiply (scalar_tensor_tensor) instead of folded into trig.
  - Causal mask folded into the PE score accumulation (trilA x maskB adds
    -1e9*count on diagonal tiles) as in v4.
  - All 4 Wo column blocks prefetched into SBUF during P3; P4 reads y_sb
    directly, so the P3->P4 transition has no DMA wait.
"""
import math

import numpy as np
import ml_dtypes

import concourse.mybir as mybir
import concourse.tile as tile
from concourse import bass_isa
from concourse import bacc
from concourse.bass_utils import run_bass_kernel_spmd

B, T, C = 4, 2048, 2048
H, D, DH = 16, 128, 64
HG = 8          # heads per core (head-group)
GD = HG * D     # group output dims = 1024
NT = T // 512   # 4 query blocks of 512
NCT = C // 128  # 16 contraction tiles
EPS = 1e-5
SCL = 1.0 / math.sqrt(D)
NEG = -1.0e9

dt = mybir.dt
AF = mybir.ActivationFunctionType
ALU = mybir.AluOpType

TWO_PI = 6.283185307179586
INV_2PI = 1.0 / TWO_PI
CW1 = float(np.float32(6.28125))
CW2 = float(np.float32(TWO_PI - 6.28125))
CW3 = float(TWO_PI - CW1 - float(np.float32(TWO_PI - 6.28125)))
MAGIC = 12582912.0  # 1.5 * 2^23: fp32 add/sub rounds to nearest int
HALF_PI = 1.5707963267948966
PI = 3.141592653589793

_CACHE = {}


def _build():
    f32, bf16 = dt.float32, dt.bfloat16
    nc = bacc.Bacc(None, target_bir_lowering=False)
    with tile.TileContext(nc) as tc:
        # weight inputs are host-prearranged to the exact SBUF layouts so
        # every DMA moves 4KB-contiguous per-partition runs
        xt_d = nc.dram_tensor("xt", (C, T), bf16, kind="ExternalInput")
        wq_d = nc.dram_tensor("wq", (HG, 128, NCT * 128), bf16,
                              kind="ExternalInput")
        wk_d = nc.dram_tensor("wk", (HG, 128, NCT * 128), bf16,
                              kind="ExternalInput")
        wv_d = nc.dram_tensor("wv", (4, 128, NCT * 256), bf16,
                              kind="ExternalInput")
        wo_d = nc.dram_tensor("wo", (128, 4 * HG * 512), bf16,
                              kind="ExternalInput")
        womg2_d = nc.dram_tensor("womg2", (128, NCT * 128), bf16,
                                 kind="ExternalInput")
        b16_d = nc.dram_tensor("b16", (1, 1), f32, kind="ExternalInput")
        freqs_d = nc.dram_tensor("freqs", (128, 1), f32, kind="ExternalInput")
        gq_d = nc.dram_tensor("gq", (128, 1), f32, kind="ExternalInput")
        gk_d = nc.dram_tensor("gk", (128, 1), f32, kind="ExternalInput")
        maskB_d = nc.dram_tensor("maskB", (128, 4 * 512), bf16, kind="ExternalInput")
        ones128_d = nc.dram_tensor("ones128", (128, 128), bf16,
                                   kind="ExternalInput")
        out_d = nc.dram_tensor("out", (T, C), f32, kind="ExternalOutput")

        with tc.tile_pool(name="const", bufs=1) as constp, \
             tc.tile_pool(name="dram", bufs=1, space="DRAM") as dramp, \
             tc.tile_pool(name="core", bufs=1) as corep, \
             tc.tile_pool(name="qkp", bufs=1) as qkp, \
             tc.tile_pool(name="psp", bufs=1, space="PSUM") as psp:

            # ---- constants ----
            b16t = constp.tile([1, 1], f32)
            nc.sync.dma_start(b16t[:], b16_d[:])
            freqs = constp.tile([128, 1], f32)
            nc.sync.dma_start(freqs[:], freqs_d[:])
            gq = constp.tile([128, 1], f32)
            nc.sync.dma_start(gq[:], gq_d[:])
            gk = constp.tile([128, 1], f32)
            nc.sync.dma_start(gk[:], gk_d[:])
            maskB = constp.tile([128, 4 * 512], bf16)
            ones128 = constp.tile([128, 128], bf16)
            nc.sync.dma_start(ones128[:], ones128_d[:])
            eps128 = constp.tile([128, 1], f32)
            nc.vector.memset(eps128[:], EPS)

            # all-heads v and y stay resident; q/k spill to DRAM (separate
            # tiles so a head's readback only waits on its own spill)
            v_sb = corep.tile([128, 4 * 16 * 256], bf16)  # (pair*16+tt)*256
            y_sb = corep.tile([128, HG * T], bf16)        # yT per head at h*T
            qk_d = {(wi, h): dramp.tile([128, T], bf16, name=f"qkd_{wi}_{h}")
                    for wi in range(2) for h in range(HG)}

            # stream q/k per head (ring 2); heads 0/1 are fetched from
            # inside P2 as soon as their spills are issued
            qh_slots = [None, None]

            def fetch_head(h):
                qh = qkp.tile([128, T], bf16, tag="qh", bufs=2,
                              name=f"qh_{h}")
                kh = qkp.tile([128, T], bf16, tag="kh", bufs=2,
                              name=f"kh_{h}")
                for c in range(2):
                    sl = slice(c * 1024, (c + 1) * 1024)
                    nc.sync.dma_start(qh[:, sl], qk_d[(0, h)][:, sl])
                    nc.sync.dma_start(kh[:, sl], qk_d[(1, h)][:, sl])
                qh_slots[h % 2] = (qh, kh)

            with tc.tile_pool(name="xtp", bufs=1) as xtp, \
                 tc.tile_pool(name="wstp", bufs=1) as wstp, \
                 tc.tile_pool(name="trigp", bufs=1) as trigp:
                trigA = trigp.tile([128, T], bf16)
                trigB = trigp.tile([128, T], bf16)
                _proj(nc, tc, xt_d, wq_d, wk_d, wv_d, womg2_d,
                      xtp, wstp, psp,
                      b16t, freqs, gq, gk, ones128, eps128,
                      trigA, trigB, v_sb, qk_d, fetch_head)

            with tc.tile_pool(name="attp", bufs=1) as attp, \
                 tc.tile_pool(name="p4w", bufs=1) as p4w, \
                 tc.tile_pool(name="p4o", bufs=1) as p4o:
                for c in range(2):
                    nc.sync.dma_start(maskB[:, c * 1024:(c + 1) * 1024],
                                      maskB_d[:, c * 1024:(c + 1) * 1024])
                wo_all = p4w.tile([128, 4 * HG * 512], bf16)  # (cb*8+hh)*512
                for cb in range(4):
                    for c in range(2):
                        sl = slice(cb * 4096 + c * 2048,
                                   cb * 4096 + (c + 1) * 2048)
                        nc.sync.dma_start(wo_all[:, sl], wo_d[:, sl])

                _attention(nc, tc, attp, psp, qh_slots, fetch_head,
                           maskB, ones128, v_sb, y_sb)

                # ---- P4: out = y^T W_o (partial over heads) ----
                for ti in range(T // 128):
                    for cb in range(4):
                        ops = psp.tile([128, 512], f32, tag="y", bufs=4,
                                       name=f"ops_{ti}_{cb}")
                        for hh in range(HG):
                            nc.tensor.matmul(
                                ops[:],
                                y_sb[:, hh * T + ti * 128:hh * T + (ti + 1) * 128],
                                wo_all[:, (cb * 8 + hh) * 512:(cb * 8 + hh + 1) * 512],
                                start=(hh == 0), stop=(hh == HG - 1))
                        osb = p4o.tile([128, 512], f32, tag="osb", bufs=4)
                        if cb % 2 == 0:
                            nc.scalar.copy(osb[:], ops[:])
                        else:
                            nc.vector.tensor_copy(osb[:], ops[:])
                        nc.sync.dma_start(
                            out_d[ti * 128:(ti + 1) * 128,
                                  cb * 512:(cb + 1) * 512],
                            osb[:])
    nc.compile()
    return nc


def _proj(nc, tc, xt_d, wq_d, wk_d, wv_d, womg2_d,
          xtp, wstp, psp,
          b16t, freqs, gq, gk, ones128, eps128,
          trigA, trigB, v_sb, qk_d, fetch_head):
    f32, bf16 = dt.float32, dt.bfloat16

    sites = [(pair, wi, hl) for pair in range(4) for wi in range(2)
             for hl in range(2)]
    wp_slots = [None, None]
    wvp_slots = [None]

    # each dma_start lands on one ~22GB/s queue: split panel transfers into
    # chunks so they spread across queues (runs stay 4KB-contiguous)
    def issue_panel(si):
        pair, wi, hl = sites[si]
        h = pair * 2 + hl
        w_d = (wq_d, wk_d)[wi]
        wp = wstp.tile([128, NCT * 128], bf16, tag="wp", bufs=2,
                       name=f"wp_{si}")
        for c in range(2):
            nc.sync.dma_start(wp[:, c * 1024:(c + 1) * 1024],
                              w_d[h, :, c * 1024:(c + 1) * 1024])
        wp_slots[si % 2] = wp

    def issue_wvp(pair):
        wvp = wstp.tile([128, NCT * 256], bf16, tag="wvp", bufs=1,
                        name=f"wvp_{pair}")
        for c in range(4):
            nc.sync.dma_start(wvp[:, c * 1024:(c + 1) * 1024],
                              wv_d[pair, :, c * 1024:(c + 1) * 1024])
        wvp_slots[0] = wvp

    # ---- P1: omega -> phi -> trig (pools closed before P2's scratch) ----
    # split by T-halves so trig for J0/J1 is ready as soon as the first
    # half of x lands; the x DMA is half-major for the same reason
    with tc.tile_pool(name="p1p", bufs=1) as p1p, \
         tc.tile_pool(name="rowp", bufs=1) as rowp:
        womg2 = p1p.tile([128, NCT * 128], bf16, name="womg2")
        for c in range(8):
            nc.sync.dma_start(womg2[:, c * 256:(c + 1) * 256],
                              womg2_d[:, c * 256:(c + 1) * 256])
        xts = xtp.tile([128, NCT * T], bf16)  # c-tile i at [i*T,(i+1)*T)
        for half in range(2):
            for i in range(NCT):
                for c2 in range(2):
                    cs = half * 1024 + c2 * 512
                    nc.sync.dma_start(
                        xts[:, i * T + cs:i * T + cs + 512],
                        xt_d[i * 128:(i + 1) * 128, cs:cs + 512])
            if half == 0:
                issue_panel(0)
        issue_wvp(0)

        HT = T // 2
        omega = rowp.tile([1, T], f32, tag="om")
        incl = rowp.tile([1, T], f32, tag="incl")
        off = rowp.tile([1, 1], f32, tag="off")

        def trig_J(J):
            sl = slice(J * 512, (J + 1) * 512)
            phi2 = p1p.tile([128, 512], f32, tag="p1", bufs=3,
                            name=f"phi2_{J}")
            nc.gpsimd.partition_broadcast(phi2[:], incl[:, sl])
            ang = p1p.tile([128, 512], f32, tag="p1", bufs=3, name=f"ang_{J}")
            # rows 64:128 of freqs are negated: sin rows come out negated,
            # cos rows unchanged (even), which is the rotation's sign layout
            nc.vector.tensor_scalar(ang[:], phi2[:], freqs[:], None,
                                    op0=ALU.mult)
            mm = p1p.tile([128, 512], f32, tag="p1", bufs=3, name=f"mm_{J}")
            nc.vector.tensor_scalar(mm[:], ang[:], INV_2PI, MAGIC,
                                    op0=ALU.mult, op1=ALU.add)
            kk = p1p.tile([128, 512], f32, tag="p1", bufs=3, name=f"kk_{J}")
            nc.vector.tensor_scalar_add(kk[:], mm[:], -MAGIC)
            red = p1p.tile([128, 512], f32, tag="p1", bufs=3, name=f"red_{J}")
            nc.vector.cody_waite_cascade(red[:], ang[:], kk[:], CW1, CW2, CW3)
            red2 = p1p.tile([128, 512], f32, tag="p1", bufs=3,
                            name=f"red2_{J}")
            nc.vector.add_range_wrap(red2[:], red[:], HALF_PI, PI, TWO_PI)
            nc.scalar.activation(trigB[:, sl], red[:], AF.Sin)
            nc.scalar.activation(trigA[:, sl], red2[:], AF.Sin)

        for half in range(2):
            hsl = slice(half * HT, (half + 1) * HT)
            for Jh in range(2):
                J = half * 2 + Jh
                omps = psp.tile([128, 512], f32, tag="y", bufs=4,
                                name=f"omps_{J}")
                for i in range(NCT):
                    nc.tensor.matmul(
                        omps[:], womg2[:, i * 128:(i + 1) * 128],
                        xts[:, i * T + J * 512:i * T + J * 512 + 512],
                        start=(i == 0), stop=(i == NCT - 1))
                nc.scalar.activation(omega[:, J * 512:(J + 1) * 512],
                                     omps[0:1, :],
                                     AF.Sigmoid, scale=1.0 / 16.0,
                                     bias=b16t[:])
            # inclusive scan of this half, then phi (in-place) = incl - omega
            nc.vector.tensor_tensor_scan(incl[:, hsl], omega[:, hsl],
                                         omega[:, hsl], 0.0,
                                         ALU.add, ALU.bypass)
            if half == 0:
                nc.vector.tensor_copy(off[:], incl[:, HT - 1:HT])
            else:
                nc.vector.tensor_scalar(incl[:, hsl], incl[:, hsl],
                                        off[:], None, op0=ALU.add)
            nc.vector.tensor_sub(incl[:, hsl], incl[:, hsl], omega[:, hsl])
            trig_J(half * 2)
            trig_J(half * 2 + 1)

    # ---- P2: q/k/v for all pairs; q/k rotated+normed then spilled ----
    pend_norm = [None]
    pend_tail = [None]

    def flush(pend):
        if pend[0] is not None:
            pend[0]()
            pend[0] = None

    with tc.tile_pool(name="scp", bufs=1) as scp:
        for pair in range(4):
            wvp = wvp_slots[0]

            # --- v first: needs no trig, so the P1 sigmoid->scan->trig
            # chain has cover before the first rotation consumer ---
            vbase = pair * 16 * 256
            for tq in range(4):
                vps = []
                for q4 in range(2):
                    vps.append(psp.tile([128, 1024], f32, tag="s", bufs=2,
                                        name=f"vps_{pair}_{tq}_{q4}"))
                for q4 in range(2):
                    for i in range(NCT):
                        for t2 in range(2):
                            t = q4 * 2 + t2
                            tt = tq * 4 + t
                            nc.tensor.matmul(
                                vps[q4][:, t2 * 512:t2 * 512 + 256],
                                xts[:, i * T + tt * 128:i * T + (tt + 1) * 128],
                                wvp[:, i * 256:(i + 1) * 256],
                                start=(i == 0), stop=(i == NCT - 1))
                for t in range(4):
                    tt = tq * 4 + t
                    # split copies ACT/DVE so neither engine's backlog
                    # stalls vps PSUM-bank reuse
                    dst = v_sb[:, vbase + tt * 256:vbase + (tt + 1) * 256]
                    src = vps[t // 2][:, (t % 2) * 512:(t % 2) * 512 + 256]
                    if t % 2 == 0 and pair < 3:
                        nc.scalar.copy(dst, src)
                    else:
                        nc.vector.tensor_copy(dst, src)
                if tq == 0:
                    flush(pend_tail)
                    flush(pend_norm)
                    if pair == 1:
                        # pair-0 spills (heads 0/1) are all issued now
                        fetch_head(0)
                        fetch_head(1)
            if pair + 1 < 4:
                issue_wvp(pair + 1)

            for wi in range(2):
                for hl in range(2):
                    si = pair * 4 + wi * 2 + hl
                    if si + 1 < len(sites):
                        issue_panel(si + 1)
                    wp = wp_slots[si % 2]
                    h = pair * 2 + hl
                    spill_d = qk_d[(wi, h)]
                    g = (gq, gk)[wi]
                    qsite = scp.tile([128, T], bf16, tag="qk", bufs=2,
                                     name=f"qsite_{si}")
                    sqs = []
                    for Jp in range(2):
                        qps2 = psp.tile([128, 1024], f32, tag="s", bufs=2,
                                        name=f"qps2_{si}_{Jp}")
                        for i in range(NCT):
                            for Jh in range(2):
                                J = 2 * Jp + Jh
                                nc.tensor.matmul(
                                    qps2[:, Jh * 512:(Jh + 1) * 512],
                                    wp[:, i * 128:(i + 1) * 128],
                                    xts[:, i * T + J * 512:i * T + J * 512 + 512],
                                    start=(i == 0), stop=(i == NCT - 1))
                        # flush prev site's ssq tail mid-stream so its rnb
                        # is ready before this site's norm
                        if Jp == 1:
                            flush(pend_tail)
                        for Jh in range(2):
                            J = 2 * Jp + Jh
                            qps = qps2[:, Jh * 512:(Jh + 1) * 512]
                            sl = slice(J * 512, (J + 1) * 512)
                            # rotation: cos part straight into qsite, then
                            # += swapped-half sin part (sign baked in trigB)
                            nc.vector.tensor_tensor(qsite[:, sl], qps,
                                                    trigA[:, sl], op=ALU.mult)
                            Bt = scp.tile([128, 512], f32, tag="rb", bufs=2,
                                          name=f"Bt_{si}_{J}")
                            nc.vector.tensor_tensor(
                                Bt[0:DH, :],
                                qps2[DH:128, Jh * 512:(Jh + 1) * 512],
                                trigB[0:DH, sl], op=ALU.mult)
                            nc.vector.tensor_tensor(
                                Bt[DH:128, :],
                                qps2[0:DH, Jh * 512:(Jh + 1) * 512],
                                trigB[DH:128, sl], op=ALU.mult)
                            nc.vector.tensor_add(
                                qsite[:, sl], qsite[:, sl], Bt[:])
                            # sum-of-squares (rotation preserves norms)
                            sq = scp.tile([128, 512], bf16, tag="sq", bufs=6,
                                          name=f"sq_{si}_{J}")
                            nc.scalar.activation(sq[:], qps, AF.Square)
                            sqs.append((J, sq))
                    flush(pend_norm)

                    def tail(sqs=tuple(sqs), si=si, qsite=qsite, g=g,
                             spill_d=spill_d, pend_norm=pend_norm):
                        rnbs = []
                        for J, sq in sqs:
                            ssqps = psp.tile([128, 512], f32, tag="y", bufs=4,
                                             name=f"ssq_{si}_{J}")
                            nc.tensor.matmul(ssqps[:], ones128[:], sq[:],
                                             start=True, stop=True)
                            rnb = scp.tile([128, 512], bf16, tag="rnb",
                                           bufs=4, name=f"rnb_{si}_{J}")
                            nc.scalar.activation(rnb[:], ssqps[:],
                                                 AF.Abs_reciprocal_sqrt,
                                                 scale=1.0 / 128.0,
                                                 bias=eps128[:])
                            rnbs.append((J, rnb))

                        def norm():
                            for J, rnb in rnbs:
                                sl = slice(J * 512, (J + 1) * 512)
                                nc.vector.scalar_tensor_tensor(
                                    qsite[:, sl], qsite[:, sl], g[:], rnb[:],
                                    op0=ALU.mult, op1=ALU.mult)
                            nc.sync.dma_start(spill_d[:], qsite[:])
                        pend_norm[0] = norm
                    pend_tail[0] = tail

        flush(pend_tail)
        flush(pend_norm)


def _attention(nc, tc, attp, psp, qh_slots, fetch_head,
               maskB, ones128, v_sb, y_sb):
    """Flat software pipeline over all (h, J) block-rows at Ip granularity.

    Per task (h, J, Ip): scores for key-tile pair Ip into a [128,1024] PSUM
    tile, ACT Exp -> ex2 bf16, 0/1 mask multiply on diagonal tiles (DVE),
    and a two-level DVE fold tree feeding a GpSimd partition_all_reduce +
    accumulate for the softmax denominator (no PE involvement).  Consumption
    lags 2 tasks: yps matmuls per ex2 half.  Row epilogue (reciprocal of the
    GpSimd-reduced denominator + y write) runs on DVE.
    """
    f32, bf16 = dt.float32, dt.bfloat16
    tasks = []
    for h in range(HG):
        # J descending: the first tasks of each head are non-diagonal, so
        # the pipeline fill never waits on the DVE mask path
        for J in reversed(range(NT)):
            for Ip in range(2 * J + 2):
                tasks.append((h, J, Ip))

    state = {}  # (h, J) -> (yps, dps)
    pend_fold = [None]
    inflight = []

    def issue(ti_t):
        ti, t = ti_t
        h, J, Ip = t
        if J == NT - 1 and Ip == 0 and 1 <= h < HG - 1:
            # heads 0/1 are prefetched from P2; ring slot h-1 frees once
            # all of head h-1's scores have issued
            fetch_head(h + 1)
        qh, kh = qh_slots[h % 2]
        sps2 = psp.tile([128, 1024], f32, tag="s", bufs=2,
                        name=f"sps_{h}_{J}_{Ip}")
        for half in range(2):
            I = 2 * Ip + half
            osl = sps2[:, half * 512:(half + 1) * 512]
            nc.tensor.matmul(
                osl,
                kh[:, I * 128:(I + 1) * 128],
                qh[:, J * 512:(J + 1) * 512],
                start=True, stop=True)
        ex2 = attp.tile([128, 1024], bf16, tag="ex", bufs=4,
                        name=f"ex_{h}_{J}_{Ip}")
        diag_r = 2 * Ip - 4 * J
        if diag_r == 2 and ti >= 4:
            # second diagonal tile: columns [0,256) are fully masked; skip
            # their exp.  The stale ring-slot contents there are old finite
            # exp values (ti>=4 skips first use), zeroed by the mask below.
            nc.scalar.activation(ex2[:, 256:1024], sps2[:, 256:1024],
                                 AF.Exp, scale=SCL)
        else:
            nc.scalar.activation(ex2[:], sps2[:], AF.Exp, scale=SCL)
        if diag_r >= 0:
            # causal mask: zero the upper-triangular part of the two
            # diagonal key tiles with one in-place 0/1 multiply (DVE)
            nc.vector.tensor_tensor(ex2[:], ex2[:],
                                    maskB[:, diag_r * 512:diag_r * 512 + 1024],
                                    op=ALU.mult)
        fold = attp.tile([128, 512], bf16, tag="fold", bufs=4,
                         name=f"fold_{h}_{J}_{Ip}")
        nc.vector.tensor_add(fold[:], ex2[:, 0:512], ex2[:, 512:1024])
        if Ip % 2 == 0:
            pend_fold[0] = fold
            dps_op = None
        else:
            # second fold level: one dps matmul per 4 key tiles
            dps_op = attp.tile([128, 512], bf16, tag="fold2", bufs=3,
                               name=f"fold2_{h}_{J}_{Ip}")
            nc.vector.tensor_add(dps_op[:], pend_fold[0][:], fold[:])
        return (t, ex2, dps_op)

    def consume(item):
        t, ex2, dps_op = item
        h, J, Ip = t
        nI = 4 * J + 4
        nIp = 2 * J + 2
        if Ip == 0:
            yps = psp.tile([128, 512], f32, tag="y", bufs=4,
                           name=f"yps_{h}_{J}")
            dps = psp.tile([128, 512], f32, tag="y", bufs=4,
                           name=f"dps_{h}_{J}")
            state[(h, J)] = (yps, dps)
        yps, dps = state[(h, J)]
        vbase = (h // 2) * 16 * 256
        hoff = (h % 2) * 128
        for half in range(2):
            I = 2 * Ip + half
            nc.tensor.matmul(
                yps[:],
                v_sb[:, vbase + I * 256 + hoff:vbase + I * 256 + hoff + 128],
                ex2[:, half * 512:(half + 1) * 512],
                start=(I == 0), stop=(I == nI - 1))
        if dps_op is not None:
            nc.tensor.matmul(dps[:], ones128[:], dps_op[:],
                             start=(Ip == 1), stop=(Ip == nIp - 1))
        if Ip == nIp - 1:
            rb = attp.tile([128, 512], f32, tag="rbc", bufs=2,
                           name=f"rb_{h}_{J}")
            nc.vector.reciprocal_approx_fast(out=rb[:], in_=dps[:])
            nc.vector.tensor_tensor(
                y_sb[:, h * T + J * 512:h * T + (J + 1) * 512],
                yps[:], rb[:], op=ALU.mult)
            del state[(h, J)]

    LAG = 2
    for ti, t in enumerate(tasks):
        inflight.append(issue((ti, t)))
        if len(inflight) > LAG:
            consume(inflight.pop(0))
    while inflight:
        consume(inflight.pop(0))


def _host_prep(inputs):
    bf = ml_dtypes.bfloat16
    x = np.asarray(inputs["x"], dtype=np.float32)
    Wq = np.asarray(inputs["Wq"], dtype=np.float32)
    Wk = np.asarray(inputs["Wk"], dtype=np.float32)
    Wv = np.asarray(inputs["Wv"], dtype=np.float32)
    Wo = np.asarray(inputs["Wo"], dtype=np.float32)
    w_omega = np.asarray(inputs["w_omega"], dtype=np.float32)
    b_omega = np.asarray(inputs["b_omega"], dtype=np.float32)
    log_freq = np.asarray(inputs["log_freq"], dtype=np.float32)
    q_gamma = np.asarray(inputs["q_gamma"], dtype=np.float32)
    k_gamma = np.asarray(inputs["k_gamma"], dtype=np.float32)

    womg = w_omega.reshape(NCT, 128).T.astype(np.float32)
    # replicated across output rows: womg2[:, i*128+c] = w_omega[i*128+:] col c
    womg2 = np.repeat(womg.T[:, :, None], 128, axis=2)  # [i, 128k, 128c]
    womg2 = womg2.transpose(1, 0, 2).reshape(128, NCT * 128).astype(bf)
    b16 = (b_omega / 16.0).reshape(1, 1).astype(np.float32)
    f = np.exp(log_freq)
    freqs = np.concatenate([f, -f]).reshape(128, 1).astype(np.float32)
    gqv = q_gamma.reshape(128, 1).astype(np.float32)
    gkv = k_gamma.reshape(128, 1).astype(np.float32)
    ones128 = np.ones((128, 128), dtype=bf)
    p = np.arange(128)[:, None]
    c = np.arange(512)[None, :]
    # 0/1 keep-mask for the diagonal key tiles: key p + r*128 <= query c
    maskB = np.concatenate(
        [((p + r * 128) <= c).astype(np.float32) for r in range(4)],
        axis=1).astype(bf)

    def panels_qk(W, g):
        # [h, p, i*128+m] = W_core_T[i*128+p, h*128+m]
        WT = W[g * GD:(g + 1) * GD, :].T  # [C, GD]
        A = WT.reshape(NCT, 128, HG, 128).transpose(2, 1, 0, 3)
        return np.ascontiguousarray(A.reshape(HG, 128, NCT * 128)).astype(bf)

    def panels_v(W, g):
        # [pair, p, i*256+n] = W_core_T[i*128+p, pair*256+n]
        WT = W[g * GD:(g + 1) * GD, :].T
        A = WT.reshape(NCT, 128, 4, 256).transpose(2, 1, 0, 3)
        return np.ascontiguousarray(A.reshape(4, 128, NCT * 256)).astype(bf)

    def panel_o(W, g):
        # [p, (cb*8+hh)*512+c] = W_core_T[hh*128+p, cb*512+c]
        WT = W[:, g * GD:(g + 1) * GD].T  # [GD, C]
        A = WT.reshape(HG, 128, 4, 512).transpose(1, 2, 0, 3)
        return np.ascontiguousarray(A.reshape(128, 4 * HG * 512)).astype(bf)

    in_maps = []
    for core in range(8):
        b, g = core // 2, core % 2
        in_maps.append({
            "xt": np.ascontiguousarray(x[b].T).astype(bf),
            "wq": panels_qk(Wq, g),
            "wk": panels_qk(Wk, g),
            "wv": panels_v(Wv, g),
            "wo": panel_o(Wo, g),
            "womg2": womg2, "b16": b16,
            "freqs": freqs,
            "gq": gqv, "gk": gkv,
            "maskB": maskB, "ones128": ones128,
        })
    return in_maps


def kernel(**inputs) -> np.ndarray:
    if "nc" not in _CACHE:
        _CACHE["nc"] = _build()
    nc = _CACHE["nc"]
    in_maps = _host_prep(inputs)
    res = run_bass_kernel_spmd(nc, in_maps, core_ids=list(range(8)))
    out = np.empty((B, T, C), dtype=np.float32)
    for b in range(B):
        out[b] = res.results[2 * b]["out"] + res.results[2 * b + 1]["out"]
    return out


# revision 35
# speedup vs baseline: 1.0463x; 1.0261x over previous
"""Trainium2 Bass kernel for causal self-attention with cumulative-phase rotary
embedding (nn_CausalSelfAttention_64338610094602).

Sharding: 8 cores = 4 batches x 2 head-groups (tensor-parallel over heads).
Each core computes, for its (batch, 8-head group):
  omega/phi (replicated per batch), QKV projections, rotation + RMSNorm,
  causal attention (transposed-scores layout, max-free softmax), and a
  partial output projection. Host sums the two head-group partials per batch.

v5 design notes (vs v4's per-pair phases):
  - All projections first (P1 omega/trig, P2 all 4 pairs' q/k/v), then one
    flat attention pipeline over all 32 (head, J) block-rows, then P4.
    The PE instruction stream never alternates sections, which avoids both
    the per-row ACT-latency bubbles and the p-state ramp (PE runs at 1.2GHz
    for 3us after any idle gap, 2.4GHz only when continuously busy).
  - q/k (all 8 heads, post-norm, bf16) spill to DRAM during P2 and stream
    back per-head in P3 (SBUF cannot hold 8 heads of q+k next to xts);
    v and y stay SBUF-resident for all heads (no y round-trip).
  - Softmax denominator: each ex2 [128,1024] tile is folded to [128,512]
    on DVE (bf16 add of the two key-tile halves) and the PE ones-matmul
    runs on the folded tile -- half the PE columns of v4's dps.
  - Rotation sign baked into the frequency vector (rows 64:128 negative)
    so trig tiles are written straight out of ACT Sin; gamma applied in the
    RMSNorm multiply (scalar_tensor_tensor) instead of folded into trig.
  - Causal mask folded into the PE score accumulation (trilA x maskB adds
    -1e9*count on diagonal tiles) as in v4.
  - All 4 Wo column blocks prefetched into SBUF during P3; P4 reads y_sb
    directly, so the P3->P4 transition has no DMA wait.
"""
import math

import numpy as np
import ml_dtypes

import concourse.mybir as mybir
import concourse.tile as tile
from concourse import bass_isa
from concourse import bacc
from concourse.bass_utils import run_bass_kernel_spmd

B, T, C = 4, 2048, 2048
H, D, DH = 16, 128, 64
HG = 8          # heads per core (head-group)
GD = HG * D     # group output dims = 1024
NT = T // 512   # 4 query blocks of 512
NCT = C // 128  # 16 contraction tiles
EPS = 1e-5
SCL = 1.0 / math.sqrt(D)
NEG = -1.0e9

dt = mybir.dt
AF = mybir.ActivationFunctionType
ALU = mybir.AluOpType

TWO_PI = 6.283185307179586
INV_2PI = 1.0 / TWO_PI
CW1 = float(np.float32(6.28125))
CW2 = float(np.float32(TWO_PI - 6.28125))
CW3 = float(TWO_PI - CW1 - float(np.float32(TWO_PI - 6.28125)))
MAGIC = 12582912.0  # 1.5 * 2^23: fp32 add/sub rounds to nearest int
HALF_PI = 1.5707963267948966
PI = 3.141592653589793

_CACHE = {}


def _build():
    f32, bf16 = dt.float32, dt.bfloat16
    nc = bacc.Bacc(None, target_bir_lowering=False)
    with tile.TileContext(nc) as tc:
        # weight inputs are host-prearranged to the exact SBUF layouts so
        # every DMA moves 4KB-contiguous per-partition runs
        xt_d = nc.dram_tensor("xt", (C, T), bf16, kind="ExternalInput")
        wq_d = nc.dram_tensor("wq", (HG, 128, NCT * 128), bf16,
                              kind="ExternalInput")
        wk_d = nc.dram_tensor("wk", (HG, 128, NCT * 128), bf16,
                              kind="ExternalInput")
        wv_d = nc.dram_tensor("wv", (4, 128, NCT * 256), bf16,
                              kind="ExternalInput")
        wo_d = nc.dram_tensor("wo", (128, 4 * HG * 512), bf16,
                              kind="ExternalInput")
        womg2_d = nc.dram_tensor("womg2", (128, NCT * 128), bf16,
                                 kind="ExternalInput")
        b16_d = nc.dram_tensor("b16", (1, 1), f32, kind="ExternalInput")
        freqs_d = nc.dram_tensor("freqs", (128, 1), f32, kind="ExternalInput")
        gq_d = nc.dram_tensor("gq", (128, 1), f32, kind="ExternalInput")
        gk_d = nc.dram_tensor("gk", (128, 1), f32, kind="ExternalInput")
        maskB_d = nc.dram_tensor("maskB", (128, 4 * 512), bf16, kind="ExternalInput")
        ones128_d = nc.dram_tensor("ones128", (128, 128), bf16,
                                   kind="ExternalInput")
        out_d = nc.dram_tensor("out", (T, C), f32, kind="ExternalOutput")

        with tc.tile_pool(name="const", bufs=1) as constp, \
             tc.tile_pool(name="dram", bufs=1, space="DRAM") as dramp, \
             tc.tile_pool(name="core", bufs=1) as corep, \
             tc.tile_pool(name="qkp", bufs=1) as qkp, \
             tc.tile_pool(name="psp", bufs=1, space="PSUM") as psp:

            # ---- constants ----
            b16t = constp.tile([1, 1], f32)
            nc.sync.dma_start(b16t[:], b16_d[:])
            freqs = constp.tile([128, 1], f32)
            nc.sync.dma_start(freqs[:], freqs_d[:])
            gq = constp.tile([128, 1], f32)
            nc.sync.dma_start(gq[:], gq_d[:])
            gk = constp.tile([128, 1], f32)
            nc.sync.dma_start(gk[:], gk_d[:])
            maskB = constp.tile([128, 4 * 512], bf16)
            ones128 = constp.tile([128, 128], bf16)
            nc.sync.dma_start(ones128[:], ones128_d[:])
            eps128 = constp.tile([128, 1], f32)
            nc.vector.memset(eps128[:], EPS)

            # all-heads v and y stay resident; q/k spill to DRAM (separate
            # tiles so a head's readback only waits on its own spill)
            v_sb = corep.tile([128, 4 * 16 * 256], bf16)  # (pair*16+tt)*256
            y_sb = corep.tile([128, HG * T], bf16)        # yT per head at h*T
            qk_d = {(wi, h): dramp.tile([128, T], bf16, name=f"qkd_{wi}_{h}")
                    for wi in range(2) for h in range(HG)}

            # stream q/k per head (ring 2); heads 0/1 are fetched from
            # inside P2 as soon as their spills are issued
            qh_slots = [None, None]

            def fetch_head(h):
                qh = qkp.tile([128, T], bf16, tag="qh", bufs=2,
                              name=f"qh_{h}")
                kh = qkp.tile([128, T], bf16, tag="kh", bufs=2,
                              name=f"kh_{h}")
                for c in range(2):
                    sl = slice(c * 1024, (c + 1) * 1024)
                    nc.sync.dma_start(qh[:, sl], qk_d[(0, h)][:, sl])
                    nc.sync.dma_start(kh[:, sl], qk_d[(1, h)][:, sl])
                qh_slots[h % 2] = (qh, kh)

            with tc.tile_pool(name="xtp", bufs=1) as xtp, \
                 tc.tile_pool(name="wstp", bufs=1) as wstp, \
                 tc.tile_pool(name="trigp", bufs=1) as trigp:
                trigA = trigp.tile([128, T], bf16)
                trigB = trigp.tile([128, T], bf16)
                _proj(nc, tc, xt_d, wq_d, wk_d, wv_d, womg2_d,
                      xtp, wstp, psp,
                      b16t, freqs, gq, gk, ones128, eps128,
                      trigA, trigB, v_sb, qk_d, fetch_head)

            with tc.tile_pool(name="attp", bufs=1) as attp, \
                 tc.tile_pool(name="p4w", bufs=1) as p4w, \
                 tc.tile_pool(name="p4o", bufs=1) as p4o:
                for c in range(2):
                    nc.sync.dma_start(maskB[:, c * 1024:(c + 1) * 1024],
                                      maskB_d[:, c * 1024:(c + 1) * 1024])
                wo_all = p4w.tile([128, 4 * HG * 512], bf16)  # (cb*8+hh)*512
                for cb in range(4):
                    for c in range(2):
                        sl = slice(cb * 4096 + c * 2048,
                                   cb * 4096 + (c + 1) * 2048)
                        nc.sync.dma_start(wo_all[:, sl], wo_d[:, sl])

                _attention(nc, tc, attp, psp, qh_slots, fetch_head,
                           maskB, ones128, v_sb, y_sb)

                # ---- P4: out = y^T W_o (partial over heads) ----
                for ti in range(T // 128):
                    for cb in range(4):
                        ops = psp.tile([128, 512], f32, tag="y", bufs=4,
                                       name=f"ops_{ti}_{cb}")
                        for hh in range(HG):
                            nc.tensor.matmul(
                                ops[:],
                                y_sb[:, hh * T + ti * 128:hh * T + (ti + 1) * 128],
                                wo_all[:, (cb * 8 + hh) * 512:(cb * 8 + hh + 1) * 512],
                                start=(hh == 0), stop=(hh == HG - 1))
                        osb = p4o.tile([128, 512], f32, tag="osb", bufs=4)
                        if cb % 2 == 0:
                            nc.scalar.copy(osb[:], ops[:])
                        else:
                            nc.vector.tensor_copy(osb[:], ops[:])
                        nc.sync.dma_start(
                            out_d[ti * 128:(ti + 1) * 128,
                                  cb * 512:(cb + 1) * 512],
                            osb[:])
    nc.compile()
    return nc


def _proj(nc, tc, xt_d, wq_d, wk_d, wv_d, womg2_d,
          xtp, wstp, psp,
          b16t, freqs, gq, gk, ones128, eps128,
          trigA, trigB, v_sb, qk_d, fetch_head):
    f32, bf16 = dt.float32, dt.bfloat16

    sites = [(pair, wi, hl) for pair in range(4) for wi in range(2)
             for hl in range(2)]
    wp_slots = [None, None]
    wvp_slots = [None]

    # each dma_start lands on one ~22GB/s queue: split panel transfers into
    # chunks so they spread across queues (runs stay 4KB-contiguous)
    def issue_panel(si):
        pair, wi, hl = sites[si]
        h = pair * 2 + hl
        w_d = (wq_d, wk_d)[wi]
        wp = wstp.tile([128, NCT * 128], bf16, tag="wp", bufs=2,
                       name=f"wp_{si}")
        for c in range(2):
            nc.sync.dma_start(wp[:, c * 1024:(c + 1) * 1024],
                              w_d[h, :, c * 1024:(c + 1) * 1024])
        wp_slots[si % 2] = wp

    def issue_wvp(pair):
        wvp = wstp.tile([128, NCT * 256], bf16, tag="wvp", bufs=1,
                        name=f"wvp_{pair}")
        for c in range(4):
            nc.sync.dma_start(wvp[:, c * 1024:(c + 1) * 1024],
                              wv_d[pair, :, c * 1024:(c + 1) * 1024])
        wvp_slots[0] = wvp

    # ---- P1: omega -> phi -> trig (pools closed before P2's scratch) ----
    # split by T-halves so trig for J0/J1 is ready as soon as the first
    # half of x lands; the x DMA is half-major for the same reason
    with tc.tile_pool(name="p1p", bufs=1) as p1p, \
         tc.tile_pool(name="rowp", bufs=1) as rowp:
        womg2 = p1p.tile([128, NCT * 128], bf16, name="womg2")
        for c in range(8):
            nc.sync.dma_start(womg2[:, c * 256:(c + 1) * 256],
                              womg2_d[:, c * 256:(c + 1) * 256])
        xts = xtp.tile([128, NCT * T], bf16)  # c-tile i at [i*T,(i+1)*T)
        for half in range(2):
            for i in range(NCT):
                cs = half * 1024
                nc.sync.dma_start(
                    xts[:, i * T + cs:i * T + cs + 1024],
                    xt_d[i * 128:(i + 1) * 128, cs:cs + 1024])
            if half == 0:
                issue_panel(0)
        issue_wvp(0)

        HT = T // 2
        omega = rowp.tile([1, T], f32, tag="om")
        incl = rowp.tile([1, T], f32, tag="incl")
        off = rowp.tile([1, 1], f32, tag="off")

        def trig_J(J):
            sl = slice(J * 512, (J + 1) * 512)
            phi2 = p1p.tile([128, 512], f32, tag="p1", bufs=3,
                            name=f"phi2_{J}")
            nc.gpsimd.partition_broadcast(phi2[:], incl[:, sl])
            ang = p1p.tile([128, 512], f32, tag="p1", bufs=3, name=f"ang_{J}")
            # rows 64:128 of freqs are negated: sin rows come out negated,
            # cos rows unchanged (even), which is the rotation's sign layout
            nc.vector.tensor_scalar(ang[:], phi2[:], freqs[:], None,
                                    op0=ALU.mult)
            mm = p1p.tile([128, 512], f32, tag="p1", bufs=3, name=f"mm_{J}")
            nc.vector.tensor_scalar(mm[:], ang[:], INV_2PI, MAGIC,
                                    op0=ALU.mult, op1=ALU.add)
            kk = p1p.tile([128, 512], f32, tag="p1", bufs=3, name=f"kk_{J}")
            nc.vector.tensor_scalar_add(kk[:], mm[:], -MAGIC)
            red = p1p.tile([128, 512], f32, tag="p1", bufs=3, name=f"red_{J}")
            nc.vector.cody_waite_cascade(red[:], ang[:], kk[:], CW1, CW2, CW3)
            red2 = p1p.tile([128, 512], f32, tag="p1", bufs=3,
                            name=f"red2_{J}")
            nc.vector.add_range_wrap(red2[:], red[:], HALF_PI, PI, TWO_PI)
            nc.scalar.activation(trigB[:, sl], red[:], AF.Sin)
            nc.scalar.activation(trigA[:, sl], red2[:], AF.Sin)

        for half in range(2):
            hsl = slice(half * HT, (half + 1) * HT)
            for Jh in range(2):
                J = half * 2 + Jh
                omps = psp.tile([128, 512], f32, tag="y", bufs=4,
                                name=f"omps_{J}")
                for i in range(NCT):
                    nc.tensor.matmul(
                        omps[:], womg2[:, i * 128:(i + 1) * 128],
                        xts[:, i * T + J * 512:i * T + J * 512 + 512],
                        start=(i == 0), stop=(i == NCT - 1))
                nc.scalar.activation(omega[:, J * 512:(J + 1) * 512],
                                     omps[0:1, :],
                                     AF.Sigmoid, scale=1.0 / 16.0,
                                     bias=b16t[:])
            # inclusive scan of this half, then phi (in-place) = incl - omega
            nc.vector.tensor_tensor_scan(incl[:, hsl], omega[:, hsl],
                                         omega[:, hsl], 0.0,
                                         ALU.add, ALU.bypass)
            if half == 0:
                nc.vector.tensor_copy(off[:], incl[:, HT - 1:HT])
            else:
                nc.vector.tensor_scalar(incl[:, hsl], incl[:, hsl],
                                        off[:], None, op0=ALU.add)
            nc.vector.tensor_sub(incl[:, hsl], incl[:, hsl], omega[:, hsl])
            trig_J(half * 2)
            trig_J(half * 2 + 1)

    # ---- P2: q/k/v for all pairs; q/k rotated+normed then spilled ----
    pend_norm = [None]
    pend_tail = [None]

    def flush(pend):
        if pend[0] is not None:
            pend[0]()
            pend[0] = None

    with tc.tile_pool(name="scp", bufs=1) as scp:
        for pair in range(4):
            wvp = wvp_slots[0]

            # --- v first: needs no trig, so the P1 sigmoid->scan->trig
            # chain has cover before the first rotation consumer ---
            vbase = pair * 16 * 256
            for tq in range(4):
                vps = []
                for q4 in range(2):
                    vps.append(psp.tile([128, 1024], f32, tag="s", bufs=2,
                                        name=f"vps_{pair}_{tq}_{q4}"))
                for q4 in range(2):
                    for i in range(NCT):
                        for t2 in range(2):
                            t = q4 * 2 + t2
                            tt = tq * 4 + t
                            nc.tensor.matmul(
                                vps[q4][:, t2 * 512:t2 * 512 + 256],
                                xts[:, i * T + tt * 128:i * T + (tt + 1) * 128],
                                wvp[:, i * 256:(i + 1) * 256],
                                start=(i == 0), stop=(i == NCT - 1))
                for t in range(4):
                    tt = tq * 4 + t
                    # split copies ACT/DVE so neither engine's backlog
                    # stalls vps PSUM-bank reuse
                    dst = v_sb[:, vbase + tt * 256:vbase + (tt + 1) * 256]
                    src = vps[t // 2][:, (t % 2) * 512:(t % 2) * 512 + 256]
                    if t % 2 == 0 and pair < 3:
                        nc.scalar.copy(dst, src)
                    else:
                        nc.vector.tensor_copy(dst, src)
                if tq == 0:
                    flush(pend_tail)
                    flush(pend_norm)
                    if pair == 1:
                        # pair-0 spills (heads 0/1) are all issued now
                        fetch_head(0)
                        fetch_head(1)
            if pair + 1 < 4:
                issue_wvp(pair + 1)

            for wi in range(2):
                for hl in range(2):
                    si = pair * 4 + wi * 2 + hl
                    if si + 1 < len(sites):
                        issue_panel(si + 1)
                    wp = wp_slots[si % 2]
                    h = pair * 2 + hl
                    spill_d = qk_d[(wi, h)]
                    g = (gq, gk)[wi]
                    qsite = scp.tile([128, T], bf16, tag="qk", bufs=2,
                                     name=f"qsite_{si}")
                    sqs = []
                    for Jp in range(2):
                        qps2 = psp.tile([128, 1024], f32, tag="s", bufs=2,
                                        name=f"qps2_{si}_{Jp}")
                        for i in range(NCT):
                            for Jh in range(2):
                                J = 2 * Jp + Jh
                                nc.tensor.matmul(
                                    qps2[:, Jh * 512:(Jh + 1) * 512],
                                    wp[:, i * 128:(i + 1) * 128],
                                    xts[:, i * T + J * 512:i * T + J * 512 + 512],
                                    start=(i == 0), stop=(i == NCT - 1))
                        # flush prev site's ssq tail mid-stream so its rnb
                        # is ready before this site's norm
                        if Jp == 1:
                            flush(pend_tail)
                        for Jh in range(2):
                            J = 2 * Jp + Jh
                            qps = qps2[:, Jh * 512:(Jh + 1) * 512]
                            sl = slice(J * 512, (J + 1) * 512)
                            # rotation: cos part straight into qsite, then
                            # += swapped-half sin part (sign baked in trigB)
                            nc.vector.tensor_tensor(qsite[:, sl], qps,
                                                    trigA[:, sl], op=ALU.mult)
                            Bt = scp.tile([128, 512], f32, tag="rb", bufs=2,
                                          name=f"Bt_{si}_{J}")
                            nc.vector.tensor_tensor(
                                Bt[0:DH, :],
                                qps2[DH:128, Jh * 512:(Jh + 1) * 512],
                                trigB[0:DH, sl], op=ALU.mult)
                            nc.vector.tensor_tensor(
                                Bt[DH:128, :],
                                qps2[0:DH, Jh * 512:(Jh + 1) * 512],
                                trigB[DH:128, sl], op=ALU.mult)
                            nc.vector.tensor_add(
                                qsite[:, sl], qsite[:, sl], Bt[:])
                            # sum-of-squares (rotation preserves norms)
                            sq = scp.tile([128, 512], bf16, tag="sq", bufs=6,
                                          name=f"sq_{si}_{J}")
                            nc.scalar.activation(sq[:], qps, AF.Square)
                            sqs.append((J, sq))
                    flush(pend_norm)

                    def tail(sqs=tuple(sqs), si=si, qsite=qsite, g=g,
                             spill_d=spill_d, pend_norm=pend_norm):
                        rnbs = []
                        for J, sq in sqs:
                            ssqps = psp.tile([128, 512], f32, tag="y", bufs=4,
                                             name=f"ssq_{si}_{J}")
                            nc.tensor.matmul(ssqps[:], ones128[:], sq[:],
                                             start=True, stop=True)
                            rnb = scp.tile([128, 512], bf16, tag="rnb",
                                           bufs=4, name=f"rnb_{si}_{J}")
                            nc.scalar.activation(rnb[:], ssqps[:],
                                                 AF.Abs_reciprocal_sqrt,
                                                 scale=1.0 / 128.0,
                                                 bias=eps128[:])
                            rnbs.append((J, rnb))

                        def norm():
                            for J, rnb in rnbs:
                                sl = slice(J * 512, (J + 1) * 512)
                                nc.vector.scalar_tensor_tensor(
                                    qsite[:, sl], qsite[:, sl], g[:], rnb[:],
                                    op0=ALU.mult, op1=ALU.mult)
                            nc.sync.dma_start(spill_d[:], qsite[:])
                        pend_norm[0] = norm
                    pend_tail[0] = tail

        flush(pend_tail)
        flush(pend_norm)


def _attention(nc, tc, attp, psp, qh_slots, fetch_head,
               maskB, ones128, v_sb, y_sb):
    """Flat software pipeline over all (h, J) block-rows at Ip granularity.

    Per task (h, J, Ip): scores for key-tile pair Ip into a [128,1024] PSUM
    tile, ACT Exp -> ex2 bf16, 0/1 mask multiply on diagonal tiles (DVE),
    and a two-level DVE fold tree feeding a GpSimd partition_all_reduce +
    accumulate for the softmax denominator (no PE involvement).  Consumption
    lags 2 tasks: yps matmuls per ex2 half.  Row epilogue (reciprocal of the
    GpSimd-reduced denominator + y write) runs on DVE.
    """
    f32, bf16 = dt.float32, dt.bfloat16
    tasks = []
    for h in range(HG):
        # J descending: the first tasks of each head are non-diagonal, so
        # the pipeline fill never waits on the DVE mask path
        for J in reversed(range(NT)):
            for Ip in range(2 * J + 2):
                tasks.append((h, J, Ip))

    state = {}  # (h, J) -> (yps, dps)
    pend_fold = [None]
    inflight = []

    def issue(ti_t):
        ti, t = ti_t
        h, J, Ip = t
        if J == NT - 1 and Ip == 0 and 1 <= h < HG - 1:
            # heads 0/1 are prefetched from P2; ring slot h-1 frees once
            # all of head h-1's scores have issued
            fetch_head(h + 1)
        qh, kh = qh_slots[h % 2]
        sps2 = psp.tile([128, 1024], f32, tag="s", bufs=2,
                        name=f"sps_{h}_{J}_{Ip}")
        for half in range(2):
            I = 2 * Ip + half
            osl = sps2[:, half * 512:(half + 1) * 512]
            nc.tensor.matmul(
                osl,
                kh[:, I * 128:(I + 1) * 128],
                qh[:, J * 512:(J + 1) * 512],
                start=True, stop=True)
        ex2 = attp.tile([128, 1024], bf16, tag="ex", bufs=4,
                        name=f"ex_{h}_{J}_{Ip}")
        diag_r = 2 * Ip - 4 * J
        if diag_r == 2 and ti >= 4:
            # second diagonal tile: columns [0,256) are fully masked; skip
            # their exp.  The stale ring-slot contents there are old finite
            # exp values (ti>=4 skips first use), zeroed by the mask below.
            nc.scalar.activation(ex2[:, 256:1024], sps2[:, 256:1024],
                                 AF.Exp, scale=SCL)
        else:
            nc.scalar.activation(ex2[:], sps2[:], AF.Exp, scale=SCL)
        if diag_r >= 0:
            # causal mask: zero the upper-triangular part of the two
            # diagonal key tiles with one in-place 0/1 multiply (DVE)
            nc.vector.tensor_tensor(ex2[:], ex2[:],
                                    maskB[:, diag_r * 512:diag_r * 512 + 1024],
                                    op=ALU.mult)
        fold = attp.tile([128, 512], bf16, tag="fold", bufs=4,
                         name=f"fold_{h}_{J}_{Ip}")
        nc.vector.tensor_add(fold[:], ex2[:, 0:512], ex2[:, 512:1024])
        if Ip % 2 == 0:
            pend_fold[0] = fold
            dps_op = None
        else:
            # second fold level: one dps matmul per 4 key tiles
            dps_op = attp.tile([128, 512], bf16, tag="fold2", bufs=3,
                               name=f"fold2_{h}_{J}_{Ip}")
            nc.vector.tensor_add(dps_op[:], pend_fold[0][:], fold[:])
        return (t, ex2, dps_op)

    def consume(item):
        t, ex2, dps_op = item
        h, J, Ip = t
        nI = 4 * J + 4
        nIp = 2 * J + 2
        if Ip == 0:
            yps = psp.tile([128, 512], f32, tag="y", bufs=4,
                           name=f"yps_{h}_{J}")
            dps = psp.tile([128, 512], f32, tag="y", bufs=4,
                           name=f"dps_{h}_{J}")
            state[(h, J)] = (yps, dps)
        yps, dps = state[(h, J)]
        vbase = (h // 2) * 16 * 256
        hoff = (h % 2) * 128
        for half in range(2):
            I = 2 * Ip + half
            nc.tensor.matmul(
                yps[:],
                v_sb[:, vbase + I * 256 + hoff:vbase + I * 256 + hoff + 128],
                ex2[:, half * 512:(half + 1) * 512],
                start=(I == 0), stop=(I == nI - 1))
        if dps_op is not None:
            nc.tensor.matmul(dps[:], ones128[:], dps_op[:],
                             start=(Ip == 1), stop=(Ip == nIp - 1))
        if Ip == nIp - 1:
            rb = attp.tile([128, 512], f32, tag="rbc", bufs=2,
                           name=f"rb_{h}_{J}")
            nc.vector.reciprocal_approx_fast(out=rb[:], in_=dps[:])
            nc.vector.tensor_tensor(
                y_sb[:, h * T + J * 512:h * T + (J + 1) * 512],
                yps[:], rb[:], op=ALU.mult)
            del state[(h, J)]

    LAG = 2
    for ti, t in enumerate(tasks):
        inflight.append(issue((ti, t)))
        if len(inflight) > LAG:
            consume(inflight.pop(0))
    while inflight:
        consume(inflight.pop(0))


def _host_prep(inputs):
    bf = ml_dtypes.bfloat16
    x = np.asarray(inputs["x"], dtype=np.float32)
    Wq = np.asarray(inputs["Wq"], dtype=np.float32)
    Wk = np.asarray(inputs["Wk"], dtype=np.float32)
    Wv = np.asarray(inputs["Wv"], dtype=np.float32)
    Wo = np.asarray(inputs["Wo"], dtype=np.float32)
    w_omega = np.asarray(inputs["w_omega"], dtype=np.float32)
    b_omega = np.asarray(inputs["b_omega"], dtype=np.float32)
    log_freq = np.asarray(inputs["log_freq"], dtype=np.float32)
    q_gamma = np.asarray(inputs["q_gamma"], dtype=np.float32)
    k_gamma = np.asarray(inputs["k_gamma"], dtype=np.float32)

    womg = w_omega.reshape(NCT, 128).T.astype(np.float32)
    # replicated across output rows: womg2[:, i*128+c] = w_omega[i*128+:] col c
    womg2 = np.repeat(womg.T[:, :, None], 128, axis=2)  # [i, 128k, 128c]
    womg2 = womg2.transpose(1, 0, 2).reshape(128, NCT * 128).astype(bf)
    b16 = (b_omega / 16.0).reshape(1, 1).astype(np.float32)
    f = np.exp(log_freq)
    freqs = np.concatenate([f, -f]).reshape(128, 1).astype(np.float32)
    gqv = q_gamma.reshape(128, 1).astype(np.float32)
    gkv = k_gamma.reshape(128, 1).astype(np.float32)
    ones128 = np.ones((128, 128), dtype=bf)
    p = np.arange(128)[:, None]
    c = np.arange(512)[None, :]
    # 0/1 keep-mask for the diagonal key tiles: key p + r*128 <= query c
    maskB = np.concatenate(
        [((p + r * 128) <= c).astype(np.float32) for r in range(4)],
        axis=1).astype(bf)

    def panels_qk(W, g):
        # [h, p, i*128+m] = W_core_T[i*128+p, h*128+m]
        WT = W[g * GD:(g + 1) * GD, :].T  # [C, GD]
        A = WT.reshape(NCT, 128, HG, 128).transpose(2, 1, 0, 3)
        return np.ascontiguousarray(A.reshape(HG, 128, NCT * 128)).astype(bf)

    def panels_v(W, g):
        # [pair, p, i*256+n] = W_core_T[i*128+p, pair*256+n]
        WT = W[g * GD:(g + 1) * GD, :].T
        A = WT.reshape(NCT, 128, 4, 256).transpose(2, 1, 0, 3)
        return np.ascontiguousarray(A.reshape(4, 128, NCT * 256)).astype(bf)

    def panel_o(W, g):
        # [p, (cb*8+hh)*512+c] = W_core_T[hh*128+p, cb*512+c]
        WT = W[:, g * GD:(g + 1) * GD].T  # [GD, C]
        A = WT.reshape(HG, 128, 4, 512).transpose(1, 2, 0, 3)
        return np.ascontiguousarray(A.reshape(128, 4 * HG * 512)).astype(bf)

    in_maps = []
    for core in range(8):
        b, g = core // 2, core % 2
        in_maps.append({
            "xt": np.ascontiguousarray(x[b].T).astype(bf),
            "wq": panels_qk(Wq, g),
            "wk": panels_qk(Wk, g),
            "wv": panels_v(Wv, g),
            "wo": panel_o(Wo, g),
            "womg2": womg2, "b16": b16,
            "freqs": freqs,
            "gq": gqv, "gk": gkv,
            "maskB": maskB, "ones128": ones128,
        })
    return in_maps


def kernel(**inputs) -> np.ndarray:
    if "nc" not in _CACHE:
        _CACHE["nc"] = _build()
    nc = _CACHE["nc"]
    in_maps = _host_prep(inputs)
    res = run_bass_kernel_spmd(nc, in_maps, core_ids=list(range(8)))
    out = np.empty((B, T, C), dtype=np.float32)
    for b in range(B):
        out[b] = res.results[2 * b]["out"] + res.results[2 * b + 1]["out"]
    return out


# revision 36
# speedup vs baseline: 1.0520x; 1.0055x over previous
"""Trainium2 Bass kernel for causal self-attention with cumulative-phase rotary
embedding (nn_CausalSelfAttention_64338610094602).

Sharding: 8 cores = 4 batches x 2 head-groups (tensor-parallel over heads).
Each core computes, for its (batch, 8-head group):
  omega/phi (replicated per batch), QKV projections, rotation + RMSNorm,
  causal attention (transposed-scores layout, max-free softmax), and a
  partial output projection. Host sums the two head-group partials per batch.

v5 design notes (vs v4's per-pair phases):
  - All projections first (P1 omega/trig, P2 all 4 pairs' q/k/v), then one
    flat attention pipeline over all 32 (head, J) block-rows, then P4.
    The PE instruction stream never alternates sections, which avoids both
    the per-row ACT-latency bubbles and the p-state ramp (PE runs at 1.2GHz
    for 3us after any idle gap, 2.4GHz only when continuously busy).
  - q/k (all 8 heads, post-norm, bf16) spill to DRAM during P2 and stream
    back per-head in P3 (SBUF cannot hold 8 heads of q+k next to xts);
    v and y stay SBUF-resident for all heads (no y round-trip).
  - Softmax denominator: each ex2 [128,1024] tile is folded to [128,512]
    on DVE (bf16 add of the two key-tile halves) and the PE ones-matmul
    runs on the folded tile -- half the PE columns of v4's dps.
  - Rotation sign baked into the frequency vector (rows 64:128 negative)
    so trig tiles are written straight out of ACT Sin; gamma applied in the
    RMSNorm multiply (scalar_tensor_tensor) instead of folded into trig.
  - Causal mask folded into the PE score accumulation (trilA x maskB adds
    -1e9*count on diagonal tiles) as in v4.
  - All 4 Wo column blocks prefetched into SBUF during P3; P4 reads y_sb
    directly, so the P3->P4 transition has no DMA wait.
"""
import math

import numpy as np
import ml_dtypes

import concourse.mybir as mybir
import concourse.tile as tile
from concourse import bass_isa
from concourse import bacc
from concourse.bass_utils import run_bass_kernel_spmd

B, T, C = 4, 2048, 2048
H, D, DH = 16, 128, 64
HG = 8          # heads per core (head-group)
GD = HG * D     # group output dims = 1024
NT = T // 512   # 4 query blocks of 512
NCT = C // 128  # 16 contraction tiles
EPS = 1e-5
SCL = 1.0 / math.sqrt(D)
NEG = -1.0e9

dt = mybir.dt
AF = mybir.ActivationFunctionType
ALU = mybir.AluOpType

TWO_PI = 6.283185307179586
INV_2PI = 1.0 / TWO_PI
CW1 = float(np.float32(6.28125))
CW2 = float(np.float32(TWO_PI - 6.28125))
CW3 = float(TWO_PI - CW1 - float(np.float32(TWO_PI - 6.28125)))
MAGIC = 12582912.0  # 1.5 * 2^23: fp32 add/sub rounds to nearest int
HALF_PI = 1.5707963267948966
PI = 3.141592653589793

_CACHE = {}


def _build():
    f32, bf16 = dt.float32, dt.bfloat16
    nc = bacc.Bacc(None, target_bir_lowering=False)
    with tile.TileContext(nc) as tc:
        # weight inputs are host-prearranged to the exact SBUF layouts so
        # every DMA moves 4KB-contiguous per-partition runs
        xt_d = nc.dram_tensor("xt", (C, T), bf16, kind="ExternalInput")
        wq_d = nc.dram_tensor("wq", (HG, 128, NCT * 128), bf16,
                              kind="ExternalInput")
        wk_d = nc.dram_tensor("wk", (HG, 128, NCT * 128), bf16,
                              kind="ExternalInput")
        wv_d = nc.dram_tensor("wv", (4, 128, NCT * 256), bf16,
                              kind="ExternalInput")
        wo_d = nc.dram_tensor("wo", (128, 4 * HG * 512), bf16,
                              kind="ExternalInput")
        womg2_d = nc.dram_tensor("womg2", (128, NCT * 128), bf16,
                                 kind="ExternalInput")
        b16_d = nc.dram_tensor("b16", (1, 1), f32, kind="ExternalInput")
        freqs_d = nc.dram_tensor("freqs", (128, 1), f32, kind="ExternalInput")
        gq_d = nc.dram_tensor("gq", (128, 1), f32, kind="ExternalInput")
        gk_d = nc.dram_tensor("gk", (128, 1), f32, kind="ExternalInput")
        maskB_d = nc.dram_tensor("maskB", (128, 4 * 512), bf16, kind="ExternalInput")
        ones128_d = nc.dram_tensor("ones128", (128, 128), bf16,
                                   kind="ExternalInput")
        out_d = nc.dram_tensor("out", (T, C), f32, kind="ExternalOutput")

        with tc.tile_pool(name="const", bufs=1) as constp, \
             tc.tile_pool(name="dram", bufs=1, space="DRAM") as dramp, \
             tc.tile_pool(name="core", bufs=1) as corep, \
             tc.tile_pool(name="qkp", bufs=1) as qkp, \
             tc.tile_pool(name="psp", bufs=1, space="PSUM") as psp:

            # ---- constants ----
            b16t = constp.tile([1, 1], f32)
            nc.sync.dma_start(b16t[:], b16_d[:])
            freqs = constp.tile([128, 1], f32)
            nc.sync.dma_start(freqs[:], freqs_d[:])
            gq = constp.tile([128, 1], f32)
            nc.sync.dma_start(gq[:], gq_d[:])
            gk = constp.tile([128, 1], f32)
            nc.sync.dma_start(gk[:], gk_d[:])
            maskB = constp.tile([128, 4 * 512], bf16)
            ones128 = constp.tile([128, 128], bf16)
            nc.sync.dma_start(ones128[:], ones128_d[:])
            eps128 = constp.tile([128, 1], f32)
            nc.vector.memset(eps128[:], EPS)

            # all-heads v and y stay resident; q/k spill to DRAM (separate
            # tiles so a head's readback only waits on its own spill)
            v_sb = corep.tile([128, 4 * 16 * 256], bf16)  # (pair*16+tt)*256
            y_sb = corep.tile([128, HG * T], bf16)        # yT per head at h*T
            qk_d = {(wi, h): dramp.tile([128, T], bf16, name=f"qkd_{wi}_{h}")
                    for wi in range(2) for h in range(HG)}

            # stream q/k per head (ring 2); heads 0/1 are fetched from
            # inside P2 as soon as their spills are issued
            qh_slots = [None, None]

            def fetch_head(h):
                qh = qkp.tile([128, T], bf16, tag="qh", bufs=2,
                              name=f"qh_{h}")
                kh = qkp.tile([128, T], bf16, tag="kh", bufs=2,
                              name=f"kh_{h}")
                for c in range(2):
                    sl = slice(c * 1024, (c + 1) * 1024)
                    nc.sync.dma_start(qh[:, sl], qk_d[(0, h)][:, sl])
                    nc.sync.dma_start(kh[:, sl], qk_d[(1, h)][:, sl])
                qh_slots[h % 2] = (qh, kh)

            with tc.tile_pool(name="xtp", bufs=1) as xtp, \
                 tc.tile_pool(name="wstp", bufs=1) as wstp, \
                 tc.tile_pool(name="trigp", bufs=1) as trigp:
                trigA = trigp.tile([128, T], bf16)
                trigB = trigp.tile([128, T], bf16)
                _proj(nc, tc, xt_d, wq_d, wk_d, wv_d, womg2_d,
                      xtp, wstp, psp,
                      b16t, freqs, gq, gk, ones128, eps128,
                      trigA, trigB, v_sb, qk_d, fetch_head)

            with tc.tile_pool(name="attp", bufs=1) as attp, \
                 tc.tile_pool(name="p4w", bufs=1) as p4w, \
                 tc.tile_pool(name="p4o", bufs=1) as p4o:
                for c in range(2):
                    nc.sync.dma_start(maskB[:, c * 1024:(c + 1) * 1024],
                                      maskB_d[:, c * 1024:(c + 1) * 1024])
                wo_all = p4w.tile([128, 4 * HG * 512], bf16)  # (cb*8+hh)*512
                for cb in range(4):
                    for c in range(2):
                        sl = slice(cb * 4096 + c * 2048,
                                   cb * 4096 + (c + 1) * 2048)
                        nc.sync.dma_start(wo_all[:, sl], wo_d[:, sl])

                _attention(nc, tc, attp, psp, qh_slots, fetch_head,
                           maskB, ones128, v_sb, y_sb)

                # ---- P4: out = y^T W_o (partial over heads) ----
                for ti in range(T // 128):
                    for cb in range(4):
                        ops = psp.tile([128, 512], f32, tag="y", bufs=4,
                                       name=f"ops_{ti}_{cb}")
                        for hh in range(HG):
                            nc.tensor.matmul(
                                ops[:],
                                y_sb[:, hh * T + ti * 128:hh * T + (ti + 1) * 128],
                                wo_all[:, (cb * 8 + hh) * 512:(cb * 8 + hh + 1) * 512],
                                start=(hh == 0), stop=(hh == HG - 1))
                        osb = p4o.tile([128, 512], f32, tag="osb", bufs=4)
                        if cb % 2 == 0:
                            nc.scalar.copy(osb[:], ops[:])
                        else:
                            nc.vector.tensor_copy(osb[:], ops[:])
                        nc.sync.dma_start(
                            out_d[ti * 128:(ti + 1) * 128,
                                  cb * 512:(cb + 1) * 512],
                            osb[:])
    nc.compile()
    return nc


def _proj(nc, tc, xt_d, wq_d, wk_d, wv_d, womg2_d,
          xtp, wstp, psp,
          b16t, freqs, gq, gk, ones128, eps128,
          trigA, trigB, v_sb, qk_d, fetch_head):
    f32, bf16 = dt.float32, dt.bfloat16

    sites = [(pair, wi, hl) for pair in range(4) for wi in range(2)
             for hl in range(2)]
    wp_slots = [None, None]
    wvp_slots = [None]

    # each dma_start lands on one ~22GB/s queue: split panel transfers into
    # chunks so they spread across queues (runs stay 4KB-contiguous)
    def issue_panel(si):
        pair, wi, hl = sites[si]
        h = pair * 2 + hl
        w_d = (wq_d, wk_d)[wi]
        wp = wstp.tile([128, NCT * 128], bf16, tag="wp", bufs=2,
                       name=f"wp_{si}")
        for c in range(2):
            nc.sync.dma_start(wp[:, c * 1024:(c + 1) * 1024],
                              w_d[h, :, c * 1024:(c + 1) * 1024])
        wp_slots[si % 2] = wp

    def issue_wvp(pair):
        wvp = wstp.tile([128, NCT * 256], bf16, tag="wvp", bufs=1,
                        name=f"wvp_{pair}")
        for c in range(4):
            nc.sync.dma_start(wvp[:, c * 1024:(c + 1) * 1024],
                              wv_d[pair, :, c * 1024:(c + 1) * 1024])
        wvp_slots[0] = wvp

    # ---- P1: omega -> phi -> trig (pools closed before P2's scratch) ----
    # split by T-halves so trig for J0/J1 is ready as soon as the first
    # half of x lands; the x DMA is half-major for the same reason
    with tc.tile_pool(name="p1p", bufs=1) as p1p, \
         tc.tile_pool(name="rowp", bufs=1) as rowp:
        womg2 = p1p.tile([128, NCT * 128], bf16, name="womg2")
        for c in range(4):
            nc.sync.dma_start(womg2[:, c * 512:(c + 1) * 512],
                              womg2_d[:, c * 512:(c + 1) * 512])
        xts = xtp.tile([128, NCT * T], bf16)  # c-tile i at [i*T,(i+1)*T)
        for half in range(2):
            for i in range(NCT):
                cs = half * 1024
                nc.sync.dma_start(
                    xts[:, i * T + cs:i * T + cs + 1024],
                    xt_d[i * 128:(i + 1) * 128, cs:cs + 1024])
            if half == 0:
                issue_panel(0)
        issue_wvp(0)

        HT = T // 2
        omega = rowp.tile([1, T], f32, tag="om")
        incl = rowp.tile([1, T], f32, tag="incl")
        off = rowp.tile([1, 1], f32, tag="off")

        def trig_J(J):
            sl = slice(J * 512, (J + 1) * 512)
            phi2 = p1p.tile([128, 512], f32, tag="p1", bufs=3,
                            name=f"phi2_{J}")
            nc.gpsimd.partition_broadcast(phi2[:], incl[:, sl])
            ang = p1p.tile([128, 512], f32, tag="p1", bufs=3, name=f"ang_{J}")
            # rows 64:128 of freqs are negated: sin rows come out negated,
            # cos rows unchanged (even), which is the rotation's sign layout
            nc.vector.tensor_scalar(ang[:], phi2[:], freqs[:], None,
                                    op0=ALU.mult)
            mm = p1p.tile([128, 512], f32, tag="p1", bufs=3, name=f"mm_{J}")
            nc.vector.tensor_scalar(mm[:], ang[:], INV_2PI, MAGIC,
                                    op0=ALU.mult, op1=ALU.add)
            kk = p1p.tile([128, 512], f32, tag="p1", bufs=3, name=f"kk_{J}")
            nc.vector.tensor_scalar_add(kk[:], mm[:], -MAGIC)
            red = p1p.tile([128, 512], f32, tag="p1", bufs=3, name=f"red_{J}")
            nc.vector.cody_waite_cascade(red[:], ang[:], kk[:], CW1, CW2, CW3)
            red2 = p1p.tile([128, 512], f32, tag="p1", bufs=3,
                            name=f"red2_{J}")
            nc.vector.add_range_wrap(red2[:], red[:], HALF_PI, PI, TWO_PI)
            nc.scalar.activation(trigB[:, sl], red[:], AF.Sin)
            nc.scalar.activation(trigA[:, sl], red2[:], AF.Sin)

        for half in range(2):
            hsl = slice(half * HT, (half + 1) * HT)
            for Jh in range(2):
                J = half * 2 + Jh
                omps = psp.tile([128, 512], f32, tag="y", bufs=4,
                                name=f"omps_{J}")
                for i in range(NCT):
                    nc.tensor.matmul(
                        omps[:], womg2[:, i * 128:(i + 1) * 128],
                        xts[:, i * T + J * 512:i * T + J * 512 + 512],
                        start=(i == 0), stop=(i == NCT - 1))
                nc.scalar.activation(omega[:, J * 512:(J + 1) * 512],
                                     omps[0:1, :],
                                     AF.Sigmoid, scale=1.0 / 16.0,
                                     bias=b16t[:])
            # inclusive scan of this half, then phi (in-place) = incl - omega
            nc.vector.tensor_tensor_scan(incl[:, hsl], omega[:, hsl],
                                         omega[:, hsl], 0.0,
                                         ALU.add, ALU.bypass)
            if half == 0:
                nc.vector.tensor_copy(off[:], incl[:, HT - 1:HT])
            else:
                nc.vector.tensor_scalar(incl[:, hsl], incl[:, hsl],
                                        off[:], None, op0=ALU.add)
            nc.vector.tensor_sub(incl[:, hsl], incl[:, hsl], omega[:, hsl])
            trig_J(half * 2)
            trig_J(half * 2 + 1)

    # ---- P2: q/k/v for all pairs; q/k rotated+normed then spilled ----
    pend_norm = [None]
    pend_tail = [None]

    def flush(pend):
        if pend[0] is not None:
            pend[0]()
            pend[0] = None

    with tc.tile_pool(name="scp", bufs=1) as scp:
        for pair in range(4):
            wvp = wvp_slots[0]

            # --- v first: needs no trig, so the P1 sigmoid->scan->trig
            # chain has cover before the first rotation consumer ---
            vbase = pair * 16 * 256
            for tq in range(4):
                vps = []
                for q4 in range(2):
                    vps.append(psp.tile([128, 1024], f32, tag="s", bufs=2,
                                        name=f"vps_{pair}_{tq}_{q4}"))
                for q4 in range(2):
                    for i in range(NCT):
                        for t2 in range(2):
                            t = q4 * 2 + t2
                            tt = tq * 4 + t
                            nc.tensor.matmul(
                                vps[q4][:, t2 * 512:t2 * 512 + 256],
                                xts[:, i * T + tt * 128:i * T + (tt + 1) * 128],
                                wvp[:, i * 256:(i + 1) * 256],
                                start=(i == 0), stop=(i == NCT - 1))
                for t in range(4):
                    tt = tq * 4 + t
                    # split copies ACT/DVE so neither engine's backlog
                    # stalls vps PSUM-bank reuse
                    dst = v_sb[:, vbase + tt * 256:vbase + (tt + 1) * 256]
                    src = vps[t // 2][:, (t % 2) * 512:(t % 2) * 512 + 256]
                    if t % 2 == 0:
                        nc.scalar.copy(dst, src)
                    else:
                        nc.vector.tensor_copy(dst, src)
                if tq == 0:
                    flush(pend_tail)
                    flush(pend_norm)
                    if pair == 1:
                        # pair-0 spills (heads 0/1) are all issued now
                        fetch_head(0)
                        fetch_head(1)
            if pair + 1 < 4:
                issue_wvp(pair + 1)

            for wi in range(2):
                for hl in range(2):
                    si = pair * 4 + wi * 2 + hl
                    if si + 1 < len(sites):
                        issue_panel(si + 1)
                    wp = wp_slots[si % 2]
                    h = pair * 2 + hl
                    spill_d = qk_d[(wi, h)]
                    g = (gq, gk)[wi]
                    qsite = scp.tile([128, T], bf16, tag="qk", bufs=2,
                                     name=f"qsite_{si}")
                    sqs = []
                    for Jp in range(2):
                        qps2 = psp.tile([128, 1024], f32, tag="s", bufs=2,
                                        name=f"qps2_{si}_{Jp}")
                        for i in range(NCT):
                            for Jh in range(2):
                                J = 2 * Jp + Jh
                                nc.tensor.matmul(
                                    qps2[:, Jh * 512:(Jh + 1) * 512],
                                    wp[:, i * 128:(i + 1) * 128],
                                    xts[:, i * T + J * 512:i * T + J * 512 + 512],
                                    start=(i == 0), stop=(i == NCT - 1))
                        # flush prev site's ssq tail mid-stream so its rnb
                        # is ready before this site's norm
                        if Jp == 1:
                            flush(pend_tail)
                        for Jh in range(2):
                            J = 2 * Jp + Jh
                            qps = qps2[:, Jh * 512:(Jh + 1) * 512]
                            sl = slice(J * 512, (J + 1) * 512)
                            # rotation: cos part straight into qsite, then
                            # += swapped-half sin part (sign baked in trigB)
                            nc.vector.tensor_tensor(qsite[:, sl], qps,
                                                    trigA[:, sl], op=ALU.mult)
                            Bt = scp.tile([128, 512], f32, tag="rb", bufs=2,
                                          name=f"Bt_{si}_{J}")
                            nc.vector.tensor_tensor(
                                Bt[0:DH, :],
                                qps2[DH:128, Jh * 512:(Jh + 1) * 512],
                                trigB[0:DH, sl], op=ALU.mult)
                            nc.vector.tensor_tensor(
                                Bt[DH:128, :],
                                qps2[0:DH, Jh * 512:(Jh + 1) * 512],
                                trigB[DH:128, sl], op=ALU.mult)
                            nc.vector.tensor_add(
                                qsite[:, sl], qsite[:, sl], Bt[:])
                            # sum-of-squares (rotation preserves norms)
                            sq = scp.tile([128, 512], bf16, tag="sq", bufs=6,
                                          name=f"sq_{si}_{J}")
                            nc.scalar.activation(sq[:], qps, AF.Square)
                            sqs.append((J, sq))
                    flush(pend_norm)

                    def tail(sqs=tuple(sqs), si=si, qsite=qsite, g=g,
                             spill_d=spill_d, pend_norm=pend_norm):
                        rnbs = []
                        for J, sq in sqs:
                            ssqps = psp.tile([128, 512], f32, tag="y", bufs=4,
                                             name=f"ssq_{si}_{J}")
                            nc.tensor.matmul(ssqps[:], ones128[:], sq[:],
                                             start=True, stop=True)
                            rnb = scp.tile([128, 512], bf16, tag="rnb",
                                           bufs=4, name=f"rnb_{si}_{J}")
                            nc.scalar.activation(rnb[:], ssqps[:],
                                                 AF.Abs_reciprocal_sqrt,
                                                 scale=1.0 / 128.0,
                                                 bias=eps128[:])
                            rnbs.append((J, rnb))

                        def norm():
                            for J, rnb in rnbs:
                                sl = slice(J * 512, (J + 1) * 512)
                                nc.vector.scalar_tensor_tensor(
                                    qsite[:, sl], qsite[:, sl], g[:], rnb[:],
                                    op0=ALU.mult, op1=ALU.mult)
                            nc.sync.dma_start(spill_d[:], qsite[:])
                        pend_norm[0] = norm
                    pend_tail[0] = tail

        flush(pend_tail)
        flush(pend_norm)


def _attention(nc, tc, attp, psp, qh_slots, fetch_head,
               maskB, ones128, v_sb, y_sb):
    """Flat software pipeline over all (h, J) block-rows at Ip granularity.

    Per task (h, J, Ip): scores for key-tile pair Ip into a [128,1024] PSUM
    tile, ACT Exp -> ex2 bf16, 0/1 mask multiply on diagonal tiles (DVE),
    and a two-level DVE fold tree feeding a GpSimd partition_all_reduce +
    accumulate for the softmax denominator (no PE involvement).  Consumption
    lags 2 tasks: yps matmuls per ex2 half.  Row epilogue (reciprocal of the
    GpSimd-reduced denominator + y write) runs on DVE.
    """
    f32, bf16 = dt.float32, dt.bfloat16
    tasks = []
    for h in range(HG):
        # J descending: the first tasks of each head are non-diagonal, so
        # the pipeline fill never waits on the DVE mask path
        for J in reversed(range(NT)):
            for Ip in range(2 * J + 2):
                tasks.append((h, J, Ip))

    state = {}  # (h, J) -> (yps, dps)
    pend_fold = [None]
    inflight = []

    def issue(ti_t):
        ti, t = ti_t
        h, J, Ip = t
        if J == NT - 1 and Ip == 0 and 1 <= h < HG - 1:
            # heads 0/1 are prefetched from P2; ring slot h-1 frees once
            # all of head h-1's scores have issued
            fetch_head(h + 1)
        qh, kh = qh_slots[h % 2]
        sps2 = psp.tile([128, 1024], f32, tag="s", bufs=2,
                        name=f"sps_{h}_{J}_{Ip}")
        for half in range(2):
            I = 2 * Ip + half
            osl = sps2[:, half * 512:(half + 1) * 512]
            nc.tensor.matmul(
                osl,
                kh[:, I * 128:(I + 1) * 128],
                qh[:, J * 512:(J + 1) * 512],
                start=True, stop=True)
        ex2 = attp.tile([128, 1024], bf16, tag="ex", bufs=4,
                        name=f"ex_{h}_{J}_{Ip}")
        diag_r = 2 * Ip - 4 * J
        if diag_r == 2 and ti >= 4:
            # second diagonal tile: columns [0,256) are fully masked; skip
            # their exp.  The stale ring-slot contents there are old finite
            # exp values (ti>=4 skips first use), zeroed by the mask below.
            nc.scalar.activation(ex2[:, 256:1024], sps2[:, 256:1024],
                                 AF.Exp, scale=SCL)
        else:
            nc.scalar.activation(ex2[:], sps2[:], AF.Exp, scale=SCL)
        if diag_r >= 0:
            # causal mask: zero the upper-triangular part of the two
            # diagonal key tiles with one in-place 0/1 multiply (DVE)
            nc.vector.tensor_tensor(ex2[:], ex2[:],
                                    maskB[:, diag_r * 512:diag_r * 512 + 1024],
                                    op=ALU.mult)
        fold = attp.tile([128, 512], bf16, tag="fold", bufs=4,
                         name=f"fold_{h}_{J}_{Ip}")
        nc.vector.tensor_add(fold[:], ex2[:, 0:512], ex2[:, 512:1024])
        if Ip % 2 == 0:
            pend_fold[0] = fold
            dps_op = None
        else:
            # second fold level: one dps matmul per 4 key tiles
            dps_op = attp.tile([128, 512], bf16, tag="fold2", bufs=3,
                               name=f"fold2_{h}_{J}_{Ip}")
            nc.vector.tensor_add(dps_op[:], pend_fold[0][:], fold[:])
        return (t, ex2, dps_op)

    def consume(item):
        t, ex2, dps_op = item
        h, J, Ip = t
        nI = 4 * J + 4
        nIp = 2 * J + 2
        if Ip == 0:
            yps = psp.tile([128, 512], f32, tag="y", bufs=4,
                           name=f"yps_{h}_{J}")
            dps = psp.tile([128, 512], f32, tag="y", bufs=4,
                           name=f"dps_{h}_{J}")
            state[(h, J)] = (yps, dps)
        yps, dps = state[(h, J)]
        vbase = (h // 2) * 16 * 256
        hoff = (h % 2) * 128
        for half in range(2):
            I = 2 * Ip + half
            nc.tensor.matmul(
                yps[:],
                v_sb[:, vbase + I * 256 + hoff:vbase + I * 256 + hoff + 128],
                ex2[:, half * 512:(half + 1) * 512],
                start=(I == 0), stop=(I == nI - 1))
        if dps_op is not None:
            nc.tensor.matmul(dps[:], ones128[:], dps_op[:],
                             start=(Ip == 1), stop=(Ip == nIp - 1))
        if Ip == nIp - 1:
            rb = attp.tile([128, 512], f32, tag="rbc", bufs=2,
                           name=f"rb_{h}_{J}")
            nc.vector.reciprocal_approx_fast(out=rb[:], in_=dps[:])
            nc.vector.tensor_tensor(
                y_sb[:, h * T + J * 512:h * T + (J + 1) * 512],
                yps[:], rb[:], op=ALU.mult)
            del state[(h, J)]

    LAG = 2
    for ti, t in enumerate(tasks):
        inflight.append(issue((ti, t)))
        if len(inflight) > LAG:
            consume(inflight.pop(0))
    while inflight:
        consume(inflight.pop(0))


def _host_prep(inputs):
    bf = ml_dtypes.bfloat16
    x = np.asarray(inputs["x"], dtype=np.float32)
    Wq = np.asarray(inputs["Wq"], dtype=np.float32)
    Wk = np.asarray(inputs["Wk"], dtype=np.float32)
    Wv = np.asarray(inputs["Wv"], dtype=np.float32)
    Wo = np.asarray(inputs["Wo"], dtype=np.float32)
    w_omega = np.asarray(inputs["w_omega"], dtype=np.float32)
    b_omega = np.asarray(inputs["b_omega"], dtype=np.float32)
    log_freq = np.asarray(inputs["log_freq"], dtype=np.float32)
    q_gamma = np.asarray(inputs["q_gamma"], dtype=np.float32)
    k_gamma = np.asarray(inputs["k_gamma"], dtype=np.float32)

    womg = w_omega.reshape(NCT, 128).T.astype(np.float32)
    # replicated across output rows: womg2[:, i*128+c] = w_omega[i*128+:] col c
    womg2 = np.repeat(womg.T[:, :, None], 128, axis=2)  # [i, 128k, 128c]
    womg2 = womg2.transpose(1, 0, 2).reshape(128, NCT * 128).astype(bf)
    b16 = (b_omega / 16.0).reshape(1, 1).astype(np.float32)
    f = np.exp(log_freq)
    freqs = np.concatenate([f, -f]).reshape(128, 1).astype(np.float32)
    gqv = q_gamma.reshape(128, 1).astype(np.float32)
    gkv = k_gamma.reshape(128, 1).astype(np.float32)
    ones128 = np.ones((128, 128), dtype=bf)
    p = np.arange(128)[:, None]
    c = np.arange(512)[None, :]
    # 0/1 keep-mask for the diagonal key tiles: key p + r*128 <= query c
    maskB = np.concatenate(
        [((p + r * 128) <= c).astype(np.float32) for r in range(4)],
        axis=1).astype(bf)

    def panels_qk(W, g):
        # [h, p, i*128+m] = W_core_T[i*128+p, h*128+m]
        WT = W[g * GD:(g + 1) * GD, :].T  # [C, GD]
        A = WT.reshape(NCT, 128, HG, 128).transpose(2, 1, 0, 3)
        return np.ascontiguousarray(A.reshape(HG, 128, NCT * 128)).astype(bf)

    def panels_v(W, g):
        # [pair, p, i*256+n] = W_core_T[i*128+p, pair*256+n]
        WT = W[g * GD:(g + 1) * GD, :].T
        A = WT.reshape(NCT, 128, 4, 256).transpose(2, 1, 0, 3)
        return np.ascontiguousarray(A.reshape(4, 128, NCT * 256)).astype(bf)

    def panel_o(W, g):
        # [p, (cb*8+hh)*512+c] = W_core_T[hh*128+p, cb*512+c]
        WT = W[:, g * GD:(g + 1) * GD].T  # [GD, C]
        A = WT.reshape(HG, 128, 4, 512).transpose(1, 2, 0, 3)
        return np.ascontiguousarray(A.reshape(128, 4 * HG * 512)).astype(bf)

    in_maps = []
    for core in range(8):
        b, g = core // 2, core % 2
        in_maps.append({
            "xt": np.ascontiguousarray(x[b].T).astype(bf),
            "wq": panels_qk(Wq, g),
            "wk": panels_qk(Wk, g),
            "wv": panels_v(Wv, g),
            "wo": panel_o(Wo, g),
            "womg2": womg2, "b16": b16,
            "freqs": freqs,
            "gq": gqv, "gk": gkv,
            "maskB": maskB, "ones128": ones128,
        })
    return in_maps


def kernel(**inputs) -> np.ndarray:
    if "nc" not in _CACHE:
        _CACHE["nc"] = _build()
    nc = _CACHE["nc"]
    in_maps = _host_prep(inputs)
    res = run_bass_kernel_spmd(nc, in_maps, core_ids=list(range(8)))
    out = np.empty((B, T, C), dtype=np.float32)
    for b in range(B):
        out[b] = res.results[2 * b]["out"] + res.results[2 * b + 1]["out"]
    return out


# revision 37
# speedup vs baseline: 1.0547x; 1.0025x over previous
"""Trainium2 Bass kernel for causal self-attention with cumulative-phase rotary
embedding (nn_CausalSelfAttention_64338610094602).

Sharding: 8 cores = 4 batches x 2 head-groups (tensor-parallel over heads).
Each core computes, for its (batch, 8-head group):
  omega/phi (replicated per batch), QKV projections, rotation + RMSNorm,
  causal attention (transposed-scores layout, max-free softmax), and a
  partial output projection. Host sums the two head-group partials per batch.

v5 design notes (vs v4's per-pair phases):
  - All projections first (P1 omega/trig, P2 all 4 pairs' q/k/v), then one
    flat attention pipeline over all 32 (head, J) block-rows, then P4.
    The PE instruction stream never alternates sections, which avoids both
    the per-row ACT-latency bubbles and the p-state ramp (PE runs at 1.2GHz
    for 3us after any idle gap, 2.4GHz only when continuously busy).
  - q/k (all 8 heads, post-norm, bf16) spill to DRAM during P2 and stream
    back per-head in P3 (SBUF cannot hold 8 heads of q+k next to xts);
    v and y stay SBUF-resident for all heads (no y round-trip).
  - Softmax denominator: each ex2 [128,1024] tile is folded to [128,512]
    on DVE (bf16 add of the two key-tile halves) and the PE ones-matmul
    runs on the folded tile -- half the PE columns of v4's dps.
  - Rotation sign baked into the frequency vector (rows 64:128 negative)
    so trig tiles are written straight out of ACT Sin; gamma applied in the
    RMSNorm multiply (scalar_tensor_tensor) instead of folded into trig.
  - Causal mask folded into the PE score accumulation (trilA x maskB adds
    -1e9*count on diagonal tiles) as in v4.
  - All 4 Wo column blocks prefetched into SBUF during P3; P4 reads y_sb
    directly, so the P3->P4 transition has no DMA wait.
"""
import math

import numpy as np
import ml_dtypes

import concourse.mybir as mybir
import concourse.tile as tile
from concourse import bass_isa
from concourse import bacc
from concourse.bass_utils import run_bass_kernel_spmd

B, T, C = 4, 2048, 2048
H, D, DH = 16, 128, 64
HG = 8          # heads per core (head-group)
GD = HG * D     # group output dims = 1024
NT = T // 512   # 4 query blocks of 512
NCT = C // 128  # 16 contraction tiles
EPS = 1e-5
SCL = 1.0 / math.sqrt(D)
NEG = -1.0e9

dt = mybir.dt
AF = mybir.ActivationFunctionType
ALU = mybir.AluOpType

TWO_PI = 6.283185307179586
INV_2PI = 1.0 / TWO_PI
CW1 = float(np.float32(6.28125))
CW2 = float(np.float32(TWO_PI - 6.28125))
CW3 = float(TWO_PI - CW1 - float(np.float32(TWO_PI - 6.28125)))
MAGIC = 12582912.0  # 1.5 * 2^23: fp32 add/sub rounds to nearest int
HALF_PI = 1.5707963267948966
PI = 3.141592653589793

_CACHE = {}


def _build():
    f32, bf16 = dt.float32, dt.bfloat16
    nc = bacc.Bacc(None, target_bir_lowering=False)
    with tile.TileContext(nc) as tc:
        # weight inputs are host-prearranged to the exact SBUF layouts so
        # every DMA moves 4KB-contiguous per-partition runs
        xt_d = nc.dram_tensor("xt", (C, T), bf16, kind="ExternalInput")
        wq_d = nc.dram_tensor("wq", (HG, 128, NCT * 128), bf16,
                              kind="ExternalInput")
        wk_d = nc.dram_tensor("wk", (HG, 128, NCT * 128), bf16,
                              kind="ExternalInput")
        wv_d = nc.dram_tensor("wv", (4, 128, NCT * 256), bf16,
                              kind="ExternalInput")
        wo_d = nc.dram_tensor("wo", (128, 4 * HG * 512), bf16,
                              kind="ExternalInput")
        womg2_d = nc.dram_tensor("womg2", (128, NCT * 128), bf16,
                                 kind="ExternalInput")
        b16_d = nc.dram_tensor("b16", (1, 1), f32, kind="ExternalInput")
        freqs_d = nc.dram_tensor("freqs", (128, 1), f32, kind="ExternalInput")
        gq_d = nc.dram_tensor("gq", (128, 1), f32, kind="ExternalInput")
        gk_d = nc.dram_tensor("gk", (128, 1), f32, kind="ExternalInput")
        maskB_d = nc.dram_tensor("maskB", (128, 4 * 512), bf16, kind="ExternalInput")
        ones128_d = nc.dram_tensor("ones128", (128, 128), bf16,
                                   kind="ExternalInput")
        out_d = nc.dram_tensor("out", (T, C), f32, kind="ExternalOutput")

        with tc.tile_pool(name="const", bufs=1) as constp, \
             tc.tile_pool(name="dram", bufs=1, space="DRAM") as dramp, \
             tc.tile_pool(name="core", bufs=1) as corep, \
             tc.tile_pool(name="qkp", bufs=1) as qkp, \
             tc.tile_pool(name="psp", bufs=1, space="PSUM") as psp:

            # ---- constants ----
            b16t = constp.tile([1, 1], f32)
            nc.sync.dma_start(b16t[:], b16_d[:])
            freqs = constp.tile([128, 1], f32)
            nc.sync.dma_start(freqs[:], freqs_d[:])
            gq = constp.tile([128, 1], f32)
            nc.sync.dma_start(gq[:], gq_d[:])
            gk = constp.tile([128, 1], f32)
            nc.sync.dma_start(gk[:], gk_d[:])
            maskB = constp.tile([128, 4 * 512], bf16)
            ones128 = constp.tile([128, 128], bf16)
            nc.sync.dma_start(ones128[:], ones128_d[:])
            eps128 = constp.tile([128, 1], f32)
            nc.vector.memset(eps128[:], EPS)

            # all-heads v and y stay resident; q/k spill to DRAM (separate
            # tiles so a head's readback only waits on its own spill)
            v_sb = corep.tile([128, 4 * 16 * 256], bf16)  # (pair*16+tt)*256
            y_sb = corep.tile([128, HG * T], bf16)        # yT per head at h*T
            qk_d = {(wi, h): dramp.tile([128, T], bf16, name=f"qkd_{wi}_{h}")
                    for wi in range(2) for h in range(HG)}

            # stream q/k per head (ring 2); heads 0/1 are fetched from
            # inside P2 as soon as their spills are issued
            qh_slots = [None, None]

            def fetch_head(h):
                qh = qkp.tile([128, T], bf16, tag="qh", bufs=2,
                              name=f"qh_{h}")
                kh = qkp.tile([128, T], bf16, tag="kh", bufs=2,
                              name=f"kh_{h}")
                for c in range(2):
                    sl = slice(c * 1024, (c + 1) * 1024)
                    nc.sync.dma_start(qh[:, sl], qk_d[(0, h)][:, sl])
                    nc.sync.dma_start(kh[:, sl], qk_d[(1, h)][:, sl])
                qh_slots[h % 2] = (qh, kh)

            with tc.tile_pool(name="xtp", bufs=1) as xtp, \
                 tc.tile_pool(name="wstp", bufs=1) as wstp, \
                 tc.tile_pool(name="trigp", bufs=1) as trigp:
                trigA = trigp.tile([128, T], bf16)
                trigB = trigp.tile([128, T], bf16)
                _proj(nc, tc, xt_d, wq_d, wk_d, wv_d, womg2_d,
                      xtp, wstp, psp,
                      b16t, freqs, gq, gk, ones128, eps128,
                      trigA, trigB, v_sb, qk_d, fetch_head)

            with tc.tile_pool(name="attp", bufs=1) as attp, \
                 tc.tile_pool(name="p4w", bufs=1) as p4w, \
                 tc.tile_pool(name="p4o", bufs=1) as p4o:
                for c in range(2):
                    nc.sync.dma_start(maskB[:, c * 1024:(c + 1) * 1024],
                                      maskB_d[:, c * 1024:(c + 1) * 1024])
                wo_all = p4w.tile([128, 4 * HG * 512], bf16)  # (cb*8+hh)*512
                for cb in range(4):
                    for c in range(2):
                        sl = slice(cb * 4096 + c * 2048,
                                   cb * 4096 + (c + 1) * 2048)
                        nc.sync.dma_start(wo_all[:, sl], wo_d[:, sl])

                _attention(nc, tc, attp, psp, qh_slots, fetch_head,
                           maskB, ones128, v_sb, y_sb)

                # ---- P4: out = y^T W_o (partial over heads) ----
                for ti in range(T // 128):
                    for cb in range(4):
                        ops = psp.tile([128, 512], f32, tag="y", bufs=4,
                                       name=f"ops_{ti}_{cb}")
                        for hh in range(HG):
                            nc.tensor.matmul(
                                ops[:],
                                y_sb[:, hh * T + ti * 128:hh * T + (ti + 1) * 128],
                                wo_all[:, (cb * 8 + hh) * 512:(cb * 8 + hh + 1) * 512],
                                start=(hh == 0), stop=(hh == HG - 1))
                        osb = p4o.tile([128, 512], f32, tag="osb", bufs=4)
                        if cb % 2 == 0:
                            nc.scalar.copy(osb[:], ops[:])
                        else:
                            nc.vector.tensor_copy(osb[:], ops[:])
                        nc.sync.dma_start(
                            out_d[ti * 128:(ti + 1) * 128,
                                  cb * 512:(cb + 1) * 512],
                            osb[:])
    nc.compile()
    return nc


def _proj(nc, tc, xt_d, wq_d, wk_d, wv_d, womg2_d,
          xtp, wstp, psp,
          b16t, freqs, gq, gk, ones128, eps128,
          trigA, trigB, v_sb, qk_d, fetch_head):
    f32, bf16 = dt.float32, dt.bfloat16

    sites = [(pair, wi, hl) for pair in range(4) for wi in range(2)
             for hl in range(2)]
    wp_slots = [None, None]
    wvp_slots = [None]

    # each dma_start lands on one ~22GB/s queue: split panel transfers into
    # chunks so they spread across queues (runs stay 4KB-contiguous)
    def issue_panel(si):
        pair, wi, hl = sites[si]
        h = pair * 2 + hl
        w_d = (wq_d, wk_d)[wi]
        wp = wstp.tile([128, NCT * 128], bf16, tag="wp", bufs=2,
                       name=f"wp_{si}")
        for c in range(2):
            nc.sync.dma_start(wp[:, c * 1024:(c + 1) * 1024],
                              w_d[h, :, c * 1024:(c + 1) * 1024])
        wp_slots[si % 2] = wp

    def issue_wvp(pair):
        wvp = wstp.tile([128, NCT * 256], bf16, tag="wvp", bufs=1,
                        name=f"wvp_{pair}")
        for c in range(4):
            nc.sync.dma_start(wvp[:, c * 1024:(c + 1) * 1024],
                              wv_d[pair, :, c * 1024:(c + 1) * 1024])
        wvp_slots[0] = wvp

    # ---- P1: omega -> phi -> trig (pools closed before P2's scratch) ----
    # split by T-halves so trig for J0/J1 is ready as soon as the first
    # half of x lands; the x DMA is half-major for the same reason
    with tc.tile_pool(name="p1p", bufs=1) as p1p, \
         tc.tile_pool(name="rowp", bufs=1) as rowp:
        womg2 = p1p.tile([128, NCT * 128], bf16, name="womg2")
        for c in range(4):
            nc.sync.dma_start(womg2[:, c * 512:(c + 1) * 512],
                              womg2_d[:, c * 512:(c + 1) * 512])
        xts = xtp.tile([128, NCT * T], bf16)  # c-tile i at [i*T,(i+1)*T)
        for half in range(2):
            for i in range(NCT):
                cs = half * 1024
                nc.sync.dma_start(
                    xts[:, i * T + cs:i * T + cs + 1024],
                    xt_d[i * 128:(i + 1) * 128, cs:cs + 1024])
            if half == 0:
                issue_panel(0)
        issue_wvp(0)

        HT = T // 2
        omega = rowp.tile([1, T], f32, tag="om")
        incl = rowp.tile([1, T], f32, tag="incl")
        off = rowp.tile([1, 1], f32, tag="off")

        def trig_J(J):
            sl = slice(J * 512, (J + 1) * 512)
            phi2 = p1p.tile([128, 512], f32, tag="p1", bufs=3,
                            name=f"phi2_{J}")
            nc.gpsimd.partition_broadcast(phi2[:], incl[:, sl])
            ang = p1p.tile([128, 512], f32, tag="p1", bufs=3, name=f"ang_{J}")
            # rows 64:128 of freqs are negated: sin rows come out negated,
            # cos rows unchanged (even), which is the rotation's sign layout
            nc.vector.tensor_scalar(ang[:], phi2[:], freqs[:], None,
                                    op0=ALU.mult)
            mm = p1p.tile([128, 512], f32, tag="p1", bufs=3, name=f"mm_{J}")
            nc.vector.tensor_scalar(mm[:], ang[:], INV_2PI, MAGIC,
                                    op0=ALU.mult, op1=ALU.add)
            kk = p1p.tile([128, 512], f32, tag="p1", bufs=3, name=f"kk_{J}")
            nc.vector.tensor_scalar_add(kk[:], mm[:], -MAGIC)
            red = p1p.tile([128, 512], f32, tag="p1", bufs=3, name=f"red_{J}")
            nc.vector.cody_waite_cascade(red[:], ang[:], kk[:], CW1, CW2, CW3)
            red2 = p1p.tile([128, 512], f32, tag="p1", bufs=3,
                            name=f"red2_{J}")
            nc.vector.add_range_wrap(red2[:], red[:], HALF_PI, PI, TWO_PI)
            nc.scalar.activation(trigB[:, sl], red[:], AF.Sin)
            nc.scalar.activation(trigA[:, sl], red2[:], AF.Sin)

        for half in range(2):
            hsl = slice(half * HT, (half + 1) * HT)
            for Jh in range(2):
                J = half * 2 + Jh
                omps = psp.tile([128, 512], f32, tag="y", bufs=4,
                                name=f"omps_{J}")
                for i in range(NCT):
                    nc.tensor.matmul(
                        omps[:], womg2[:, i * 128:(i + 1) * 128],
                        xts[:, i * T + J * 512:i * T + J * 512 + 512],
                        start=(i == 0), stop=(i == NCT - 1))
                nc.scalar.activation(omega[:, J * 512:(J + 1) * 512],
                                     omps[0:1, :],
                                     AF.Sigmoid, scale=1.0 / 16.0,
                                     bias=b16t[:])
            # inclusive scan of this half, then phi (in-place) = incl - omega
            nc.vector.tensor_tensor_scan(incl[:, hsl], omega[:, hsl],
                                         omega[:, hsl], 0.0,
                                         ALU.add, ALU.bypass)
            if half == 0:
                nc.vector.tensor_copy(off[:], incl[:, HT - 1:HT])
            else:
                nc.vector.tensor_scalar(incl[:, hsl], incl[:, hsl],
                                        off[:], None, op0=ALU.add)
            nc.vector.tensor_sub(incl[:, hsl], incl[:, hsl], omega[:, hsl])
            trig_J(half * 2)
            trig_J(half * 2 + 1)

    # ---- P2: q/k/v for all pairs; q/k rotated+normed then spilled ----
    pend_norm = [None]
    pend_tail = [None]

    def flush(pend):
        if pend[0] is not None:
            pend[0]()
            pend[0] = None

    with tc.tile_pool(name="scp", bufs=1) as scp:
        for pair in range(4):
            wvp = wvp_slots[0]

            # --- v first: needs no trig, so the P1 sigmoid->scan->trig
            # chain has cover before the first rotation consumer ---
            vbase = pair * 16 * 256
            for tq in range(4):
                vps = []
                for q4 in range(2):
                    vps.append(psp.tile([128, 1024], f32, tag="s", bufs=2,
                                        name=f"vps_{pair}_{tq}_{q4}"))
                for q4 in range(2):
                    for i in range(NCT):
                        for t2 in range(2):
                            t = q4 * 2 + t2
                            tt = tq * 4 + t
                            nc.tensor.matmul(
                                vps[q4][:, t2 * 512:t2 * 512 + 256],
                                xts[:, i * T + tt * 128:i * T + (tt + 1) * 128],
                                wvp[:, i * 256:(i + 1) * 256],
                                start=(i == 0), stop=(i == NCT - 1))
                for t in range(4):
                    tt = tq * 4 + t
                    # split copies ACT/DVE so neither engine's backlog
                    # stalls vps PSUM-bank reuse
                    dst = v_sb[:, vbase + tt * 256:vbase + (tt + 1) * 256]
                    src = vps[t // 2][:, (t % 2) * 512:(t % 2) * 512 + 256]
                    if t % 2 == 0:
                        nc.scalar.copy(dst, src)
                    else:
                        nc.vector.tensor_copy(dst, src)
                if tq == 0:
                    flush(pend_tail)
                    flush(pend_norm)
                    if pair == 1:
                        # pair-0 spills (heads 0/1) are all issued now
                        fetch_head(0)
                        fetch_head(1)
            if pair + 1 < 4:
                issue_wvp(pair + 1)

            for wi in range(2):
                for hl in range(2):
                    si = pair * 4 + wi * 2 + hl
                    if si + 1 < len(sites):
                        issue_panel(si + 1)
                    wp = wp_slots[si % 2]
                    h = pair * 2 + hl
                    spill_d = qk_d[(wi, h)]
                    g = (gq, gk)[wi]
                    qsite = scp.tile([128, T], bf16, tag="qk", bufs=2,
                                     name=f"qsite_{si}")
                    sqs = []
                    for Jp in range(2):
                        qps2 = psp.tile([128, 1024], f32, tag="s", bufs=2,
                                        name=f"qps2_{si}_{Jp}")
                        for i in range(NCT):
                            for Jh in range(2):
                                J = 2 * Jp + Jh
                                nc.tensor.matmul(
                                    qps2[:, Jh * 512:(Jh + 1) * 512],
                                    wp[:, i * 128:(i + 1) * 128],
                                    xts[:, i * T + J * 512:i * T + J * 512 + 512],
                                    start=(i == 0), stop=(i == NCT - 1))
                        # flush prev site's ssq tail mid-stream so its rnb
                        # is ready before this site's norm
                        if Jp == 1:
                            flush(pend_tail)
                        for Jh in range(2):
                            J = 2 * Jp + Jh
                            qps = qps2[:, Jh * 512:(Jh + 1) * 512]
                            sl = slice(J * 512, (J + 1) * 512)
                            # rotation: cos part straight into qsite, then
                            # += swapped-half sin part (sign baked in trigB)
                            nc.vector.tensor_tensor(qsite[:, sl], qps,
                                                    trigA[:, sl], op=ALU.mult)
                            Bt = scp.tile([128, 512], f32, tag="rb", bufs=2,
                                          name=f"Bt_{si}_{J}")
                            nc.vector.tensor_tensor(
                                Bt[0:DH, :],
                                qps2[DH:128, Jh * 512:(Jh + 1) * 512],
                                trigB[0:DH, sl], op=ALU.mult)
                            nc.vector.tensor_tensor(
                                Bt[DH:128, :],
                                qps2[0:DH, Jh * 512:(Jh + 1) * 512],
                                trigB[DH:128, sl], op=ALU.mult)
                            nc.vector.tensor_add(
                                qsite[:, sl], qsite[:, sl], Bt[:])
                            # sum-of-squares (rotation preserves norms)
                            sq = scp.tile([128, 512], bf16, tag="sq", bufs=6,
                                          name=f"sq_{si}_{J}")
                            nc.scalar.activation(sq[:], qps, AF.Square)
                            sqs.append((J, sq))
                    flush(pend_norm)

                    def tail(sqs=tuple(sqs), si=si, qsite=qsite, g=g,
                             spill_d=spill_d, pend_norm=pend_norm):
                        rnbs = []
                        for J, sq in sqs:
                            ssqps = psp.tile([128, 512], f32, tag="y", bufs=4,
                                             name=f"ssq_{si}_{J}")
                            nc.tensor.matmul(ssqps[:], ones128[:], sq[:],
                                             start=True, stop=True)
                            rnb = scp.tile([128, 512], bf16, tag="rnb",
                                           bufs=4, name=f"rnb_{si}_{J}")
                            nc.scalar.activation(rnb[:], ssqps[:],
                                                 AF.Abs_reciprocal_sqrt,
                                                 scale=1.0 / 128.0,
                                                 bias=eps128[:])
                            rnbs.append((J, rnb))

                        def norm():
                            for J, rnb in rnbs:
                                sl = slice(J * 512, (J + 1) * 512)
                                nc.vector.scalar_tensor_tensor(
                                    qsite[:, sl], qsite[:, sl], g[:], rnb[:],
                                    op0=ALU.mult, op1=ALU.mult)
                            nc.sync.dma_start(spill_d[:], qsite[:])
                        pend_norm[0] = norm
                    pend_tail[0] = tail

        flush(pend_tail)
        flush(pend_norm)


def _attention(nc, tc, attp, psp, qh_slots, fetch_head,
               maskB, ones128, v_sb, y_sb):
    """Flat software pipeline over all (h, J) block-rows at Ip granularity.

    Per task (h, J, Ip): scores for key-tile pair Ip into a [128,1024] PSUM
    tile, ACT Exp -> ex2 bf16, 0/1 mask multiply on diagonal tiles (DVE),
    and a two-level DVE fold tree feeding a GpSimd partition_all_reduce +
    accumulate for the softmax denominator (no PE involvement).  Consumption
    lags 2 tasks: yps matmuls per ex2 half.  Row epilogue (reciprocal of the
    GpSimd-reduced denominator + y write) runs on DVE.
    """
    f32, bf16 = dt.float32, dt.bfloat16
    tasks = []
    for h in range(HG):
        # J descending: the first tasks of each head are non-diagonal, so
        # the pipeline fill never waits on the DVE mask path
        for J in reversed(range(NT)):
            for Ip in range(2 * J + 2):
                tasks.append((h, J, Ip))

    state = {}  # (h, J) -> (yps, dps)
    pend_fold = [None]
    inflight = []

    def issue(ti_t):
        ti, t = ti_t
        h, J, Ip = t
        if J == NT - 1 and Ip == 0 and 1 <= h < HG - 1:
            # heads 0/1 are prefetched from P2; ring slot h-1 frees once
            # all of head h-1's scores have issued
            fetch_head(h + 1)
        qh, kh = qh_slots[h % 2]
        sps2 = psp.tile([128, 1024], f32, tag="s", bufs=2,
                        name=f"sps_{h}_{J}_{Ip}")
        for half in range(2):
            I = 2 * Ip + half
            osl = sps2[:, half * 512:(half + 1) * 512]
            nc.tensor.matmul(
                osl,
                kh[:, I * 128:(I + 1) * 128],
                qh[:, J * 512:(J + 1) * 512],
                start=True, stop=True)
        ex2 = attp.tile([128, 1024], bf16, tag="ex", bufs=6,
                        name=f"ex_{h}_{J}_{Ip}")
        diag_r = 2 * Ip - 4 * J
        if diag_r == 2 and ti >= 4:
            # second diagonal tile: columns [0,256) are fully masked; skip
            # their exp.  The stale ring-slot contents there are old finite
            # exp values (ti>=4 skips first use), zeroed by the mask below.
            nc.scalar.activation(ex2[:, 256:1024], sps2[:, 256:1024],
                                 AF.Exp, scale=SCL)
        else:
            nc.scalar.activation(ex2[:], sps2[:], AF.Exp, scale=SCL)
        if diag_r >= 0:
            # causal mask: zero the upper-triangular part of the two
            # diagonal key tiles with one in-place 0/1 multiply (DVE)
            nc.vector.tensor_tensor(ex2[:], ex2[:],
                                    maskB[:, diag_r * 512:diag_r * 512 + 1024],
                                    op=ALU.mult)
        fold = attp.tile([128, 512], bf16, tag="fold", bufs=6,
                         name=f"fold_{h}_{J}_{Ip}")
        nc.vector.tensor_add(fold[:], ex2[:, 0:512], ex2[:, 512:1024])
        if Ip % 2 == 0:
            pend_fold[0] = fold
            dps_op = None
        else:
            # second fold level: one dps matmul per 4 key tiles
            dps_op = attp.tile([128, 512], bf16, tag="fold2", bufs=4,
                               name=f"fold2_{h}_{J}_{Ip}")
            nc.vector.tensor_add(dps_op[:], pend_fold[0][:], fold[:])
        return (t, ex2, dps_op)

    def consume(item):
        t, ex2, dps_op = item
        h, J, Ip = t
        nI = 4 * J + 4
        nIp = 2 * J + 2
        if Ip == 0:
            yps = psp.tile([128, 512], f32, tag="y", bufs=4,
                           name=f"yps_{h}_{J}")
            dps = psp.tile([128, 512], f32, tag="y", bufs=4,
                           name=f"dps_{h}_{J}")
            state[(h, J)] = (yps, dps)
        yps, dps = state[(h, J)]
        vbase = (h // 2) * 16 * 256
        hoff = (h % 2) * 128
        for half in range(2):
            I = 2 * Ip + half
            nc.tensor.matmul(
                yps[:],
                v_sb[:, vbase + I * 256 + hoff:vbase + I * 256 + hoff + 128],
                ex2[:, half * 512:(half + 1) * 512],
                start=(I == 0), stop=(I == nI - 1))
        if dps_op is not None:
            nc.tensor.matmul(dps[:], ones128[:], dps_op[:],
                             start=(Ip == 1), stop=(Ip == nIp - 1))
        if Ip == nIp - 1:
            rb = attp.tile([128, 512], f32, tag="rbc", bufs=3,
                           name=f"rb_{h}_{J}")
            nc.vector.reciprocal_approx_fast(out=rb[:], in_=dps[:])
            nc.vector.tensor_tensor(
                y_sb[:, h * T + J * 512:h * T + (J + 1) * 512],
                yps[:], rb[:], op=ALU.mult)
            del state[(h, J)]

    LAG = 2
    for ti, t in enumerate(tasks):
        inflight.append(issue((ti, t)))
        if len(inflight) > LAG:
            consume(inflight.pop(0))
    while inflight:
        consume(inflight.pop(0))


def _host_prep(inputs):
    bf = ml_dtypes.bfloat16
    x = np.asarray(inputs["x"], dtype=np.float32)
    Wq = np.asarray(inputs["Wq"], dtype=np.float32)
    Wk = np.asarray(inputs["Wk"], dtype=np.float32)
    Wv = np.asarray(inputs["Wv"], dtype=np.float32)
    Wo = np.asarray(inputs["Wo"], dtype=np.float32)
    w_omega = np.asarray(inputs["w_omega"], dtype=np.float32)
    b_omega = np.asarray(inputs["b_omega"], dtype=np.float32)
    log_freq = np.asarray(inputs["log_freq"], dtype=np.float32)
    q_gamma = np.asarray(inputs["q_gamma"], dtype=np.float32)
    k_gamma = np.asarray(inputs["k_gamma"], dtype=np.float32)

    womg = w_omega.reshape(NCT, 128).T.astype(np.float32)
    # replicated across output rows: womg2[:, i*128+c] = w_omega[i*128+:] col c
    womg2 = np.repeat(womg.T[:, :, None], 128, axis=2)  # [i, 128k, 128c]
    womg2 = womg2.transpose(1, 0, 2).reshape(128, NCT * 128).astype(bf)
    b16 = (b_omega / 16.0).reshape(1, 1).astype(np.float32)
    f = np.exp(log_freq)
    freqs = np.concatenate([f, -f]).reshape(128, 1).astype(np.float32)
    gqv = q_gamma.reshape(128, 1).astype(np.float32)
    gkv = k_gamma.reshape(128, 1).astype(np.float32)
    ones128 = np.ones((128, 128), dtype=bf)
    p = np.arange(128)[:, None]
    c = np.arange(512)[None, :]
    # 0/1 keep-mask for the diagonal key tiles: key p + r*128 <= query c
    maskB = np.concatenate(
        [((p + r * 128) <= c).astype(np.float32) for r in range(4)],
        axis=1).astype(bf)

    def panels_qk(W, g):
        # [h, p, i*128+m] = W_core_T[i*128+p, h*128+m]
        WT = W[g * GD:(g + 1) * GD, :].T  # [C, GD]
        A = WT.reshape(NCT, 128, HG, 128).transpose(2, 1, 0, 3)
        return np.ascontiguousarray(A.reshape(HG, 128, NCT * 128)).astype(bf)

    def panels_v(W, g):
        # [pair, p, i*256+n] = W_core_T[i*128+p, pair*256+n]
        WT = W[g * GD:(g + 1) * GD, :].T
        A = WT.reshape(NCT, 128, 4, 256).transpose(2, 1, 0, 3)
        return np.ascontiguousarray(A.reshape(4, 128, NCT * 256)).astype(bf)

    def panel_o(W, g):
        # [p, (cb*8+hh)*512+c] = W_core_T[hh*128+p, cb*512+c]
        WT = W[:, g * GD:(g + 1) * GD].T  # [GD, C]
        A = WT.reshape(HG, 128, 4, 512).transpose(1, 2, 0, 3)
        return np.ascontiguousarray(A.reshape(128, 4 * HG * 512)).astype(bf)

    in_maps = []
    for core in range(8):
        b, g = core // 2, core % 2
        in_maps.append({
            "xt": np.ascontiguousarray(x[b].T).astype(bf),
            "wq": panels_qk(Wq, g),
            "wk": panels_qk(Wk, g),
            "wv": panels_v(Wv, g),
            "wo": panel_o(Wo, g),
            "womg2": womg2, "b16": b16,
            "freqs": freqs,
            "gq": gqv, "gk": gkv,
            "maskB": maskB, "ones128": ones128,
        })
    return in_maps


def kernel(**inputs) -> np.ndarray:
    if "nc" not in _CACHE:
        _CACHE["nc"] = _build()
    nc = _CACHE["nc"]
    in_maps = _host_prep(inputs)
    res = run_bass_kernel_spmd(nc, in_maps, core_ids=list(range(8)))
    out = np.empty((B, T, C), dtype=np.float32)
    for b in range(B):
        out[b] = res.results[2 * b]["out"] + res.results[2 * b + 1]["out"]
    return out


# revision 38
# speedup vs baseline: 1.0566x; 1.0018x over previous
"""Trainium2 Bass kernel for causal self-attention with cumulative-phase rotary
embedding (nn_CausalSelfAttention_64338610094602).

Sharding: 8 cores = 4 batches x 2 head-groups (tensor-parallel over heads).
Each core computes, for its (batch, 8-head group):
  omega/phi (replicated per batch), QKV projections, rotation + RMSNorm,
  causal attention (transposed-scores layout, max-free softmax), and a
  partial output projection. Host sums the two head-group partials per batch.

v5 design notes (vs v4's per-pair phases):
  - All projections first (P1 omega/trig, P2 all 4 pairs' q/k/v), then one
    flat attention pipeline over all 32 (head, J) block-rows, then P4.
    The PE instruction stream never alternates sections, which avoids both
    the per-row ACT-latency bubbles and the p-state ramp (PE runs at 1.2GHz
    for 3us after any idle gap, 2.4GHz only when continuously busy).
  - q/k (all 8 heads, post-norm, bf16) spill to DRAM during P2 and stream
    back per-head in P3 (SBUF cannot hold 8 heads of q+k next to xts);
    v and y stay SBUF-resident for all heads (no y round-trip).
  - Softmax denominator: each ex2 [128,1024] tile is folded to [128,512]
    on DVE (bf16 add of the two key-tile halves) and the PE ones-matmul
    runs on the folded tile -- half the PE columns of v4's dps.
  - Rotation sign baked into the frequency vector (rows 64:128 negative)
    so trig tiles are written straight out of ACT Sin; gamma applied in the
    RMSNorm multiply (scalar_tensor_tensor) instead of folded into trig.
  - Causal mask folded into the PE score accumulation (trilA x maskB adds
    -1e9*count on diagonal tiles) as in v4.
  - All 4 Wo column blocks prefetched into SBUF during P3; P4 reads y_sb
    directly, so the P3->P4 transition has no DMA wait.
"""
import math

import numpy as np
import ml_dtypes

import concourse.mybir as mybir
import concourse.tile as tile
from concourse import bass_isa
from concourse import bacc
from concourse.bass_utils import run_bass_kernel_spmd

B, T, C = 4, 2048, 2048
H, D, DH = 16, 128, 64
HG = 8          # heads per core (head-group)
GD = HG * D     # group output dims = 1024
NT = T // 512   # 4 query blocks of 512
NCT = C // 128  # 16 contraction tiles
EPS = 1e-5
SCL = 1.0 / math.sqrt(D)
NEG = -1.0e9

dt = mybir.dt
AF = mybir.ActivationFunctionType
ALU = mybir.AluOpType

TWO_PI = 6.283185307179586
INV_2PI = 1.0 / TWO_PI
CW1 = float(np.float32(6.28125))
CW2 = float(np.float32(TWO_PI - 6.28125))
CW3 = float(TWO_PI - CW1 - float(np.float32(TWO_PI - 6.28125)))
MAGIC = 12582912.0  # 1.5 * 2^23: fp32 add/sub rounds to nearest int
HALF_PI = 1.5707963267948966
PI = 3.141592653589793

_CACHE = {}


def _build():
    f32, bf16 = dt.float32, dt.bfloat16
    nc = bacc.Bacc(None, target_bir_lowering=False)
    with tile.TileContext(nc) as tc:
        # weight inputs are host-prearranged to the exact SBUF layouts so
        # every DMA moves 4KB-contiguous per-partition runs
        xt_d = nc.dram_tensor("xt", (C, T), bf16, kind="ExternalInput")
        wq_d = nc.dram_tensor("wq", (HG, 128, NCT * 128), bf16,
                              kind="ExternalInput")
        wk_d = nc.dram_tensor("wk", (HG, 128, NCT * 128), bf16,
                              kind="ExternalInput")
        wv_d = nc.dram_tensor("wv", (4, 128, NCT * 256), bf16,
                              kind="ExternalInput")
        wo_d = nc.dram_tensor("wo", (128, 4 * HG * 512), bf16,
                              kind="ExternalInput")
        womg2_d = nc.dram_tensor("womg2", (128, NCT * 128), bf16,
                                 kind="ExternalInput")
        b16_d = nc.dram_tensor("b16", (1, 1), f32, kind="ExternalInput")
        freqs_d = nc.dram_tensor("freqs", (128, 1), f32, kind="ExternalInput")
        gq_d = nc.dram_tensor("gq", (128, 1), f32, kind="ExternalInput")
        gk_d = nc.dram_tensor("gk", (128, 1), f32, kind="ExternalInput")
        maskB_d = nc.dram_tensor("maskB", (128, 4 * 512), bf16, kind="ExternalInput")
        ones128_d = nc.dram_tensor("ones128", (128, 128), bf16,
                                   kind="ExternalInput")
        out_d = nc.dram_tensor("out", (T, C), f32, kind="ExternalOutput")

        with tc.tile_pool(name="const", bufs=1) as constp, \
             tc.tile_pool(name="dram", bufs=1, space="DRAM") as dramp, \
             tc.tile_pool(name="core", bufs=1) as corep, \
             tc.tile_pool(name="qkp", bufs=1) as qkp, \
             tc.tile_pool(name="psp", bufs=1, space="PSUM") as psp:

            # ---- constants ----
            b16t = constp.tile([1, 1], f32)
            nc.sync.dma_start(b16t[:], b16_d[:])
            freqs = constp.tile([128, 1], f32)
            nc.sync.dma_start(freqs[:], freqs_d[:])
            gq = constp.tile([128, 1], f32)
            nc.sync.dma_start(gq[:], gq_d[:])
            gk = constp.tile([128, 1], f32)
            nc.sync.dma_start(gk[:], gk_d[:])
            maskB = constp.tile([128, 4 * 512], bf16)
            ones128 = constp.tile([128, 128], bf16)
            nc.sync.dma_start(ones128[:], ones128_d[:])
            eps128 = constp.tile([128, 1], f32)
            nc.vector.memset(eps128[:], EPS)

            # all-heads v and y stay resident; q/k spill to DRAM (separate
            # tiles so a head's readback only waits on its own spill)
            v_sb = corep.tile([128, 4 * 16 * 256], bf16)  # (pair*16+tt)*256
            y_sb = corep.tile([128, HG * T], bf16)        # yT per head at h*T
            qk_d = {(wi, h): dramp.tile([128, T], bf16, name=f"qkd_{wi}_{h}")
                    for wi in range(2) for h in range(HG)}

            # stream q/k per head (ring 2); heads 0/1 are fetched from
            # inside P2 as soon as their spills are issued
            qh_slots = [None, None]

            def fetch_head(h):
                qh = qkp.tile([128, T], bf16, tag="qh", bufs=2,
                              name=f"qh_{h}")
                kh = qkp.tile([128, T], bf16, tag="kh", bufs=2,
                              name=f"kh_{h}")
                for c in range(2):
                    sl = slice(c * 1024, (c + 1) * 1024)
                    nc.sync.dma_start(qh[:, sl], qk_d[(0, h)][:, sl])
                    nc.sync.dma_start(kh[:, sl], qk_d[(1, h)][:, sl])
                qh_slots[h % 2] = (qh, kh)

            with tc.tile_pool(name="xtp", bufs=1) as xtp, \
                 tc.tile_pool(name="wstp", bufs=1) as wstp, \
                 tc.tile_pool(name="trigp", bufs=1) as trigp:
                trigA = trigp.tile([128, T], bf16)
                trigB = trigp.tile([128, T], bf16)
                _proj(nc, tc, xt_d, wq_d, wk_d, wv_d, womg2_d,
                      xtp, wstp, psp,
                      b16t, freqs, gq, gk, ones128, eps128,
                      trigA, trigB, v_sb, qk_d, fetch_head)

            with tc.tile_pool(name="attp", bufs=1) as attp, \
                 tc.tile_pool(name="p4w", bufs=1) as p4w, \
                 tc.tile_pool(name="p4o", bufs=1) as p4o:
                for c in range(2):
                    nc.sync.dma_start(maskB[:, c * 1024:(c + 1) * 1024],
                                      maskB_d[:, c * 1024:(c + 1) * 1024])
                wo_all = p4w.tile([128, 4 * HG * 512], bf16)  # (cb*8+hh)*512
                for cb in range(4):
                    for c in range(2):
                        sl = slice(cb * 4096 + c * 2048,
                                   cb * 4096 + (c + 1) * 2048)
                        nc.sync.dma_start(wo_all[:, sl], wo_d[:, sl])

                # ---- P4 blocks: out = y^T W_o (partial over heads) ----
                # injected into the attention stream as soon as head 7
                # finishes block-row J (all heads' y for ti 4J..4J+4 ready),
                # filling PE idle while ACT/DVE drain the remaining rows
                def p4_block(J):
                    for ti in range(4 * J, 4 * J + 4):
                        for cb in range(4):
                            ops = psp.tile([128, 512], f32, tag="y", bufs=4,
                                           name=f"ops_{ti}_{cb}")
                            for hh in range(HG):
                                nc.tensor.matmul(
                                    ops[:],
                                    y_sb[:, hh * T + ti * 128:hh * T + (ti + 1) * 128],
                                    wo_all[:, (cb * 8 + hh) * 512:(cb * 8 + hh + 1) * 512],
                                    start=(hh == 0), stop=(hh == HG - 1))
                            osb = p4o.tile([128, 512], f32, tag="osb", bufs=4)
                            if cb % 2 == 0:
                                nc.scalar.copy(osb[:], ops[:])
                            else:
                                nc.vector.tensor_copy(osb[:], ops[:])
                            nc.sync.dma_start(
                                out_d[ti * 128:(ti + 1) * 128,
                                      cb * 512:(cb + 1) * 512],
                                osb[:])

                _attention(nc, tc, attp, psp, qh_slots, fetch_head,
                           maskB, ones128, v_sb, y_sb, p4_block)
    nc.compile()
    return nc


def _proj(nc, tc, xt_d, wq_d, wk_d, wv_d, womg2_d,
          xtp, wstp, psp,
          b16t, freqs, gq, gk, ones128, eps128,
          trigA, trigB, v_sb, qk_d, fetch_head):
    f32, bf16 = dt.float32, dt.bfloat16

    sites = [(pair, wi, hl) for pair in range(4) for wi in range(2)
             for hl in range(2)]
    wp_slots = [None, None]
    wvp_slots = [None]

    # each dma_start lands on one ~22GB/s queue: split panel transfers into
    # chunks so they spread across queues (runs stay 4KB-contiguous)
    def issue_panel(si):
        pair, wi, hl = sites[si]
        h = pair * 2 + hl
        w_d = (wq_d, wk_d)[wi]
        wp = wstp.tile([128, NCT * 128], bf16, tag="wp", bufs=2,
                       name=f"wp_{si}")
        for c in range(2):
            nc.sync.dma_start(wp[:, c * 1024:(c + 1) * 1024],
                              w_d[h, :, c * 1024:(c + 1) * 1024])
        wp_slots[si % 2] = wp

    def issue_wvp(pair):
        wvp = wstp.tile([128, NCT * 256], bf16, tag="wvp", bufs=1,
                        name=f"wvp_{pair}")
        for c in range(4):
            nc.sync.dma_start(wvp[:, c * 1024:(c + 1) * 1024],
                              wv_d[pair, :, c * 1024:(c + 1) * 1024])
        wvp_slots[0] = wvp

    # ---- P1: omega -> phi -> trig (pools closed before P2's scratch) ----
    # split by T-halves so trig for J0/J1 is ready as soon as the first
    # half of x lands; the x DMA is half-major for the same reason
    with tc.tile_pool(name="p1p", bufs=1) as p1p, \
         tc.tile_pool(name="rowp", bufs=1) as rowp:
        womg2 = p1p.tile([128, NCT * 128], bf16, name="womg2")
        for c in range(4):
            nc.sync.dma_start(womg2[:, c * 512:(c + 1) * 512],
                              womg2_d[:, c * 512:(c + 1) * 512])
        xts = xtp.tile([128, NCT * T], bf16)  # c-tile i at [i*T,(i+1)*T)
        for half in range(2):
            for i in range(NCT):
                cs = half * 1024
                nc.sync.dma_start(
                    xts[:, i * T + cs:i * T + cs + 1024],
                    xt_d[i * 128:(i + 1) * 128, cs:cs + 1024])
            if half == 0:
                issue_panel(0)
        issue_wvp(0)

        HT = T // 2
        omega = rowp.tile([1, T], f32, tag="om")
        incl = rowp.tile([1, T], f32, tag="incl")
        off = rowp.tile([1, 1], f32, tag="off")

        def trig_J(J):
            sl = slice(J * 512, (J + 1) * 512)
            phi2 = p1p.tile([128, 512], f32, tag="p1", bufs=3,
                            name=f"phi2_{J}")
            nc.gpsimd.partition_broadcast(phi2[:], incl[:, sl])
            ang = p1p.tile([128, 512], f32, tag="p1", bufs=3, name=f"ang_{J}")
            # rows 64:128 of freqs are negated: sin rows come out negated,
            # cos rows unchanged (even), which is the rotation's sign layout
            nc.vector.tensor_scalar(ang[:], phi2[:], freqs[:], None,
                                    op0=ALU.mult)
            mm = p1p.tile([128, 512], f32, tag="p1", bufs=3, name=f"mm_{J}")
            nc.vector.tensor_scalar(mm[:], ang[:], INV_2PI, MAGIC,
                                    op0=ALU.mult, op1=ALU.add)
            kk = p1p.tile([128, 512], f32, tag="p1", bufs=3, name=f"kk_{J}")
            nc.vector.tensor_scalar_add(kk[:], mm[:], -MAGIC)
            red = p1p.tile([128, 512], f32, tag="p1", bufs=3, name=f"red_{J}")
            nc.vector.cody_waite_cascade(red[:], ang[:], kk[:], CW1, CW2, CW3)
            red2 = p1p.tile([128, 512], f32, tag="p1", bufs=3,
                            name=f"red2_{J}")
            nc.vector.add_range_wrap(red2[:], red[:], HALF_PI, PI, TWO_PI)
            nc.scalar.activation(trigB[:, sl], red[:], AF.Sin)
            nc.scalar.activation(trigA[:, sl], red2[:], AF.Sin)

        for half in range(2):
            hsl = slice(half * HT, (half + 1) * HT)
            for Jh in range(2):
                J = half * 2 + Jh
                omps = psp.tile([128, 512], f32, tag="y", bufs=4,
                                name=f"omps_{J}")
                for i in range(NCT):
                    nc.tensor.matmul(
                        omps[:], womg2[:, i * 128:(i + 1) * 128],
                        xts[:, i * T + J * 512:i * T + J * 512 + 512],
                        start=(i == 0), stop=(i == NCT - 1))
                nc.scalar.activation(omega[:, J * 512:(J + 1) * 512],
                                     omps[0:1, :],
                                     AF.Sigmoid, scale=1.0 / 16.0,
                                     bias=b16t[:])
            # inclusive scan of this half, then phi (in-place) = incl - omega
            nc.vector.tensor_tensor_scan(incl[:, hsl], omega[:, hsl],
                                         omega[:, hsl], 0.0,
                                         ALU.add, ALU.bypass)
            if half == 0:
                nc.vector.tensor_copy(off[:], incl[:, HT - 1:HT])
            else:
                nc.vector.tensor_scalar(incl[:, hsl], incl[:, hsl],
                                        off[:], None, op0=ALU.add)
            nc.vector.tensor_sub(incl[:, hsl], incl[:, hsl], omega[:, hsl])
            trig_J(half * 2)
            trig_J(half * 2 + 1)

    # ---- P2: q/k/v for all pairs; q/k rotated+normed then spilled ----
    pend_norm = [None]
    pend_tail = [None]

    def flush(pend):
        if pend[0] is not None:
            pend[0]()
            pend[0] = None

    with tc.tile_pool(name="scp", bufs=1) as scp:
        for pair in range(4):
            wvp = wvp_slots[0]

            # --- v first: needs no trig, so the P1 sigmoid->scan->trig
            # chain has cover before the first rotation consumer ---
            vbase = pair * 16 * 256
            for tq in range(4):
                vps = []
                for q4 in range(2):
                    vps.append(psp.tile([128, 1024], f32, tag="s", bufs=2,
                                        name=f"vps_{pair}_{tq}_{q4}"))
                for q4 in range(2):
                    for i in range(NCT):
                        for t2 in range(2):
                            t = q4 * 2 + t2
                            tt = tq * 4 + t
                            nc.tensor.matmul(
                                vps[q4][:, t2 * 512:t2 * 512 + 256],
                                xts[:, i * T + tt * 128:i * T + (tt + 1) * 128],
                                wvp[:, i * 256:(i + 1) * 256],
                                start=(i == 0), stop=(i == NCT - 1))
                for t in range(4):
                    tt = tq * 4 + t
                    # split copies ACT/DVE so neither engine's backlog
                    # stalls vps PSUM-bank reuse
                    dst = v_sb[:, vbase + tt * 256:vbase + (tt + 1) * 256]
                    src = vps[t // 2][:, (t % 2) * 512:(t % 2) * 512 + 256]
                    if t % 2 == 0:
                        nc.scalar.copy(dst, src)
                    else:
                        nc.vector.tensor_copy(dst, src)
                if tq == 0:
                    flush(pend_tail)
                    flush(pend_norm)
                    if pair == 1:
                        # pair-0 spills (heads 0/1) are all issued now
                        fetch_head(0)
                        fetch_head(1)
            if pair + 1 < 4:
                issue_wvp(pair + 1)

            for wi in range(2):
                for hl in range(2):
                    si = pair * 4 + wi * 2 + hl
                    if si + 1 < len(sites):
                        issue_panel(si + 1)
                    wp = wp_slots[si % 2]
                    h = pair * 2 + hl
                    spill_d = qk_d[(wi, h)]
                    g = (gq, gk)[wi]
                    qsite = scp.tile([128, T], bf16, tag="qk", bufs=2,
                                     name=f"qsite_{si}")
                    sqs = []
                    for Jp in range(2):
                        qps2 = psp.tile([128, 1024], f32, tag="s", bufs=2,
                                        name=f"qps2_{si}_{Jp}")
                        for i in range(NCT):
                            for Jh in range(2):
                                J = 2 * Jp + Jh
                                nc.tensor.matmul(
                                    qps2[:, Jh * 512:(Jh + 1) * 512],
                                    wp[:, i * 128:(i + 1) * 128],
                                    xts[:, i * T + J * 512:i * T + J * 512 + 512],
                                    start=(i == 0), stop=(i == NCT - 1))
                        # flush prev site's ssq tail mid-stream so its rnb
                        # is ready before this site's norm
                        if Jp == 1:
                            flush(pend_tail)
                        for Jh in range(2):
                            J = 2 * Jp + Jh
                            qps = qps2[:, Jh * 512:(Jh + 1) * 512]
                            sl = slice(J * 512, (J + 1) * 512)
                            # rotation: cos part straight into qsite, then
                            # += swapped-half sin part (sign baked in trigB)
                            nc.vector.tensor_tensor(qsite[:, sl], qps,
                                                    trigA[:, sl], op=ALU.mult)
                            Bt = scp.tile([128, 512], f32, tag="rb", bufs=2,
                                          name=f"Bt_{si}_{J}")
                            nc.vector.tensor_tensor(
                                Bt[0:DH, :],
                                qps2[DH:128, Jh * 512:(Jh + 1) * 512],
                                trigB[0:DH, sl], op=ALU.mult)
                            nc.vector.tensor_tensor(
                                Bt[DH:128, :],
                                qps2[0:DH, Jh * 512:(Jh + 1) * 512],
                                trigB[DH:128, sl], op=ALU.mult)
                            nc.vector.tensor_add(
                                qsite[:, sl], qsite[:, sl], Bt[:])
                            # sum-of-squares (rotation preserves norms)
                            sq = scp.tile([128, 512], bf16, tag="sq", bufs=6,
                                          name=f"sq_{si}_{J}")
                            nc.scalar.activation(sq[:], qps, AF.Square)
                            sqs.append((J, sq))
                    flush(pend_norm)

                    def tail(sqs=tuple(sqs), si=si, qsite=qsite, g=g,
                             spill_d=spill_d, pend_norm=pend_norm):
                        rnbs = []
                        for J, sq in sqs:
                            ssqps = psp.tile([128, 512], f32, tag="y", bufs=4,
                                             name=f"ssq_{si}_{J}")
                            nc.tensor.matmul(ssqps[:], ones128[:], sq[:],
                                             start=True, stop=True)
                            rnb = scp.tile([128, 512], bf16, tag="rnb",
                                           bufs=4, name=f"rnb_{si}_{J}")
                            nc.scalar.activation(rnb[:], ssqps[:],
                                                 AF.Abs_reciprocal_sqrt,
                                                 scale=1.0 / 128.0,
                                                 bias=eps128[:])
                            rnbs.append((J, rnb))

                        def norm():
                            for J, rnb in rnbs:
                                sl = slice(J * 512, (J + 1) * 512)
                                nc.vector.scalar_tensor_tensor(
                                    qsite[:, sl], qsite[:, sl], g[:], rnb[:],
                                    op0=ALU.mult, op1=ALU.mult)
                            nc.sync.dma_start(spill_d[:], qsite[:])
                        pend_norm[0] = norm
                    pend_tail[0] = tail

        flush(pend_tail)
        flush(pend_norm)


def _attention(nc, tc, attp, psp, qh_slots, fetch_head,
               maskB, ones128, v_sb, y_sb, p4_block):
    """Flat software pipeline over all (h, J) block-rows at Ip granularity.

    Per task (h, J, Ip): scores for key-tile pair Ip into a [128,1024] PSUM
    tile, ACT Exp -> ex2 bf16, 0/1 mask multiply on diagonal tiles (DVE),
    and a two-level DVE fold tree feeding a GpSimd partition_all_reduce +
    accumulate for the softmax denominator (no PE involvement).  Consumption
    lags 2 tasks: yps matmuls per ex2 half.  Row epilogue (reciprocal of the
    GpSimd-reduced denominator + y write) runs on DVE.
    """
    f32, bf16 = dt.float32, dt.bfloat16
    tasks = []
    for h in range(HG):
        # J descending: the first tasks of each head are non-diagonal, so
        # the pipeline fill never waits on the DVE mask path
        for J in reversed(range(NT)):
            for Ip in range(2 * J + 2):
                tasks.append((h, J, Ip))

    state = {}  # (h, J) -> (yps, dps)
    pend_fold = [None]
    inflight = []

    def issue(ti_t):
        ti, t = ti_t
        h, J, Ip = t
        if J == NT - 1 and Ip == 0 and 1 <= h < HG - 1:
            # heads 0/1 are prefetched from P2; ring slot h-1 frees once
            # all of head h-1's scores have issued
            fetch_head(h + 1)
        qh, kh = qh_slots[h % 2]
        sps2 = psp.tile([128, 1024], f32, tag="s", bufs=2,
                        name=f"sps_{h}_{J}_{Ip}")
        for half in range(2):
            I = 2 * Ip + half
            osl = sps2[:, half * 512:(half + 1) * 512]
            nc.tensor.matmul(
                osl,
                kh[:, I * 128:(I + 1) * 128],
                qh[:, J * 512:(J + 1) * 512],
                start=True, stop=True)
        ex2 = attp.tile([128, 1024], bf16, tag="ex", bufs=6,
                        name=f"ex_{h}_{J}_{Ip}")
        diag_r = 2 * Ip - 4 * J
        if diag_r == 2 and ti >= 4:
            # second diagonal tile: columns [0,256) are fully masked; skip
            # their exp.  The stale ring-slot contents there are old finite
            # exp values (ti>=4 skips first use), zeroed by the mask below.
            nc.scalar.activation(ex2[:, 256:1024], sps2[:, 256:1024],
                                 AF.Exp, scale=SCL)
        else:
            nc.scalar.activation(ex2[:], sps2[:], AF.Exp, scale=SCL)
        if diag_r >= 0:
            # causal mask: zero the upper-triangular part of the two
            # diagonal key tiles with one in-place 0/1 multiply (DVE)
            nc.vector.tensor_tensor(ex2[:], ex2[:],
                                    maskB[:, diag_r * 512:diag_r * 512 + 1024],
                                    op=ALU.mult)
        fold = attp.tile([128, 512], bf16, tag="fold", bufs=6,
                         name=f"fold_{h}_{J}_{Ip}")
        nc.vector.tensor_add(fold[:], ex2[:, 0:512], ex2[:, 512:1024])
        if Ip % 2 == 0:
            pend_fold[0] = fold
            dps_op = None
        else:
            # second fold level: one dps matmul per 4 key tiles
            dps_op = attp.tile([128, 512], bf16, tag="fold2", bufs=4,
                               name=f"fold2_{h}_{J}_{Ip}")
            nc.vector.tensor_add(dps_op[:], pend_fold[0][:], fold[:])
        return (t, ex2, dps_op)

    def consume(item):
        t, ex2, dps_op = item
        h, J, Ip = t
        nI = 4 * J + 4
        nIp = 2 * J + 2
        if Ip == 0:
            yps = psp.tile([128, 512], f32, tag="y", bufs=4,
                           name=f"yps_{h}_{J}")
            dps = psp.tile([128, 512], f32, tag="y", bufs=4,
                           name=f"dps_{h}_{J}")
            state[(h, J)] = (yps, dps)
        yps, dps = state[(h, J)]
        vbase = (h // 2) * 16 * 256
        hoff = (h % 2) * 128
        for half in range(2):
            I = 2 * Ip + half
            nc.tensor.matmul(
                yps[:],
                v_sb[:, vbase + I * 256 + hoff:vbase + I * 256 + hoff + 128],
                ex2[:, half * 512:(half + 1) * 512],
                start=(I == 0), stop=(I == nI - 1))
        if dps_op is not None:
            nc.tensor.matmul(dps[:], ones128[:], dps_op[:],
                             start=(Ip == 1), stop=(Ip == nIp - 1))
        if Ip == nIp - 1:
            rb = attp.tile([128, 512], f32, tag="rbc", bufs=3,
                           name=f"rb_{h}_{J}")
            nc.vector.reciprocal_approx_fast(out=rb[:], in_=dps[:])
            nc.vector.tensor_tensor(
                y_sb[:, h * T + J * 512:h * T + (J + 1) * 512],
                yps[:], rb[:], op=ALU.mult)
            del state[(h, J)]
            if h == HG - 1:
                p4_block(J)

    LAG = 2
    for ti, t in enumerate(tasks):
        inflight.append(issue((ti, t)))
        if len(inflight) > LAG:
            consume(inflight.pop(0))
    while inflight:
        consume(inflight.pop(0))


def _host_prep(inputs):
    bf = ml_dtypes.bfloat16
    x = np.asarray(inputs["x"], dtype=np.float32)
    Wq = np.asarray(inputs["Wq"], dtype=np.float32)
    Wk = np.asarray(inputs["Wk"], dtype=np.float32)
    Wv = np.asarray(inputs["Wv"], dtype=np.float32)
    Wo = np.asarray(inputs["Wo"], dtype=np.float32)
    w_omega = np.asarray(inputs["w_omega"], dtype=np.float32)
    b_omega = np.asarray(inputs["b_omega"], dtype=np.float32)
    log_freq = np.asarray(inputs["log_freq"], dtype=np.float32)
    q_gamma = np.asarray(inputs["q_gamma"], dtype=np.float32)
    k_gamma = np.asarray(inputs["k_gamma"], dtype=np.float32)

    womg = w_omega.reshape(NCT, 128).T.astype(np.float32)
    # replicated across output rows: womg2[:, i*128+c] = w_omega[i*128+:] col c
    womg2 = np.repeat(womg.T[:, :, None], 128, axis=2)  # [i, 128k, 128c]
    womg2 = womg2.transpose(1, 0, 2).reshape(128, NCT * 128).astype(bf)
    b16 = (b_omega / 16.0).reshape(1, 1).astype(np.float32)
    f = np.exp(log_freq)
    freqs = np.concatenate([f, -f]).reshape(128, 1).astype(np.float32)
    gqv = q_gamma.reshape(128, 1).astype(np.float32)
    gkv = k_gamma.reshape(128, 1).astype(np.float32)
    ones128 = np.ones((128, 128), dtype=bf)
    p = np.arange(128)[:, None]
    c = np.arange(512)[None, :]
    # 0/1 keep-mask for the diagonal key tiles: key p + r*128 <= query c
    maskB = np.concatenate(
        [((p + r * 128) <= c).astype(np.float32) for r in range(4)],
        axis=1).astype(bf)

    def panels_qk(W, g):
        # [h, p, i*128+m] = W_core_T[i*128+p, h*128+m]
        WT = W[g * GD:(g + 1) * GD, :].T  # [C, GD]
        A = WT.reshape(NCT, 128, HG, 128).transpose(2, 1, 0, 3)
        return np.ascontiguousarray(A.reshape(HG, 128, NCT * 128)).astype(bf)

    def panels_v(W, g):
        # [pair, p, i*256+n] = W_core_T[i*128+p, pair*256+n]
        WT = W[g * GD:(g + 1) * GD, :].T
        A = WT.reshape(NCT, 128, 4, 256).transpose(2, 1, 0, 3)
        return np.ascontiguousarray(A.reshape(4, 128, NCT * 256)).astype(bf)

    def panel_o(W, g):
        # [p, (cb*8+hh)*512+c] = W_core_T[hh*128+p, cb*512+c]
        WT = W[:, g * GD:(g + 1) * GD].T  # [GD, C]
        A = WT.reshape(HG, 128, 4, 512).transpose(1, 2, 0, 3)
        return np.ascontiguousarray(A.reshape(128, 4 * HG * 512)).astype(bf)

    in_maps = []
    for core in range(8):
        b, g = core // 2, core % 2
        in_maps.append({
            "xt": np.ascontiguousarray(x[b].T).astype(bf),
            "wq": panels_qk(Wq, g),
            "wk": panels_qk(Wk, g),
            "wv": panels_v(Wv, g),
            "wo": panel_o(Wo, g),
            "womg2": womg2, "b16": b16,
            "freqs": freqs,
            "gq": gqv, "gk": gkv,
            "maskB": maskB, "ones128": ones128,
        })
    return in_maps


def kernel(**inputs) -> np.ndarray:
    if "nc" not in _CACHE:
        _CACHE["nc"] = _build()
    nc = _CACHE["nc"]
    in_maps = _host_prep(inputs)
    res = run_bass_kernel_spmd(nc, in_maps, core_ids=list(range(8)))
    out = np.empty((B, T, C), dtype=np.float32)
    for b in range(B):
        out[b] = res.results[2 * b]["out"] + res.results[2 * b + 1]["out"]
    return out
